# revision 7
# baseline (speedup 1.0000x reference)
"""Cross-Spatial-Attention Trainium2 kernel (8 NeuronCores, spatial sharding).

v2: engine-balanced. TensorE keeps the QK 9-tap fused conv (transposed
layout) + gram + SA gate + projections; the depthwise work for v and
dwconv(y) moves to the Vector/GpSimd engines as per-channel
multiply-accumulate passes (tensor_scalar 4x + tensor_tensor 2x, fp16),
fed by a cheap 1x1 conv for v_pre. The v-mean needed by the stats
AllReduce is computed from window sums of v_pre (row-sum side path) so
each batch's AllReduce fires right after its QK gram; batch1's
y-depthwise stays on TensorE as filler inside the AllReduce window.
"""

import numpy as np
from contextlib import ExitStack

import concourse.bass as bass
import concourse.bacc as bacc
import concourse.tile as tile
from concourse import mybir
from concourse.bass_utils import run_bass_kernel_spmd

FP32 = mybir.dt.float32
FP16 = mybir.dt.float16
BF16 = mybir.dt.bfloat16
AF = mybir.ActivationFunctionType
ALU = mybir.AluOpType

B, C, H, W = 2, 128, 256, 256
HD, DH = 8, 16
NCORES = 8
RPC = H // NCORES            # 32 rows per core
HH, WW = RPC + 2, W + 2      # 34 x 258 halo'd band
FREE = HH * WW               # 8772
NLOC = RPC * W               # 8192 output positions per band per batch
NCH_T = NLOC // 128          # 64 transposed chunks
NCH_A = NLOC // 512          # 16 layout-A chunks
NTOT = float(H * W)          # global spatial size

# tap engine assignment: per (tensor, batch) a list of 9 entries
# 'd' = DVE ts+tt, 'g' = DVE ts + GpSimd tt, 't' = TensorE dense fold
V_TAPS = {0: list("dddddddGG"), 1: list("dddddddGG")}
Y_TAPS = {0: list("dddddddGG"), 1: list("ttttttttt")}

TMPH = NLOC                  # DVE tap chunk (full width)
TMPG = NLOC // 2             # gpsimd tap chunk


def _emit(tc, io):
    nc = tc.nc
    ctx = ExitStack()

    wpool = ctx.enter_context(tc.tile_pool(name="wpool", bufs=1))
    xpool = ctx.enter_context(tc.tile_pool(name="xpool", bufs=1))
    ypool = ctx.enter_context(tc.tile_pool(name="ypool", bufs=2))
    vppool = ctx.enter_context(tc.tile_pool(name="vppool", bufs=2))
    vpool = ctx.enter_context(tc.tile_pool(name="vpool", bufs=2))
    y2pool = ctx.enter_context(tc.tile_pool(name="y2pool", bufs=2))
    tmppool = ctx.enter_context(tc.tile_pool(name="tmppool", bufs=1))
    spool = ctx.enter_context(tc.tile_pool(name="spool", bufs=1))
    rpool = ctx.enter_context(tc.tile_pool(name="rpool", bufs=4))
    mpool = ctx.enter_context(tc.tile_pool(name="mpool", bufs=1))
    opool = ctx.enter_context(tc.tile_pool(name="opool", bufs=2))
    psA = ctx.enter_context(tc.tile_pool(name="psA", bufs=2, space="PSUM"))
    psQK = ctx.enter_context(tc.tile_pool(name="psQK", bufs=3, space="PSUM"))
    psG = ctx.enter_context(tc.tile_pool(name="psG", bufs=1, space="PSUM"))
    dpool = ctx.enter_context(tc.tile_pool(name="dram", bufs=4, space="DRAM"))

    def dma(dst, src):
        nc.sync.dma_start(out=dst, in_=src)

    def wload(name, shape, dt=BF16):
        t = wpool.tile(shape, dt, tag=name)
        dma(t[:], io[name][:])
        return t

    w9qk = wload("w9qk", [128, 9 * 256])     # tap t at cols [256t:256t+256]
    need_w9v = any(s == "t" for b in range(B) for s in V_TAPS[b])
    w9v = wload("w9v", [128, 9 * 128]) if need_w9v else None
    wyd = wload("wyd", [128, 9 * 128])       # diag taps for TE-assigned y taps
    wv1x1 = wload("wv1x1", [128, 128])       # v 1x1: [ic, oc]
    saw1t = wload("saw1t", [128, 32])
    w2rep = wload("w2rep", [128, 32])
    w3rep = wload("w3rep", [128, 1])
    spw1t = wload("spw1t", [128, 16], FP32)
    spw2t = wload("spw2t", [16, 16], FP32)
    spw3t = wload("spw3t", [16, 128], FP32)
    projt = wload("projt", [128, 128], FP32)
    dwvw = wload("dwvw", [128, 9], FP32)     # v depthwise tap weights
    dwyw = wload("dwyw", [128, 9], FP32)     # y depthwise tap weights
    consts = wload("consts", [128, 386], FP32)
    eye = consts[:, 0:128]
    bdmask = consts[:, 128:256]
    tempp = consts[:, 256:257]
    onesrow = consts[0:1, 257:385]

    xts, yts, vts, y2ts, saTs = [], [], [], [], []
    arreses = []

    # ---------------- helpers ----------------
    def sa_gate(b, yt):
        """spatial-attention gate -> saT [128, 64] (col j = chunk j)"""
        s1 = spool.tile([128, 2048], BF16, tag="s1")
        s2 = s1
        for g in range(4):
            ps1 = psA.tile([128, 512], FP32, tag="a")
            for k in range(4):
                nn = 4 * g + k
                r0 = 2 * nn
                yv = yt[:].rearrange("p (h w) -> p h w", h=HH)[
                    :, r0 + 1:r0 + 3, 1:257]
                nc.tensor.matmul(ps1[32 * k:32 * k + 32, :], saw1t[:, :], yv,
                                 start=True, stop=True,
                                 tile_position=(0, 32 * k)).annotate("mm_sa")
            if g % 2 == 0:
                nc.vector.tensor_scalar_max(s1[:, 512 * g:512 * g + 512], ps1[:, :], 0.0)
            else:
                nc.scalar.activation(s1[:, 512 * g:512 * g + 512], ps1[:, :], AF.Relu)
        for g in range(4):
            ps2 = psA.tile([128, 512], FP32, tag="a")
            for k in range(4):
                nc.tensor.matmul(ps2[32 * k:32 * k + 32, :],
                                 w2rep[32 * k:32 * k + 16, :],
                                 s1[32 * k:32 * k + 16, 512 * g:512 * g + 512],
                                 start=True, stop=True,
                                 tile_position=(32 * k, 32 * k))
            if g % 2 == 0:
                nc.vector.tensor_scalar_max(s2[:, 512 * g:512 * g + 512], ps2[:, :], 0.0)
            else:
                nc.scalar.activation(s2[:, 512 * g:512 * g + 512], ps2[:, :], AF.Relu)
        saT_ps = psQK.tile([128, 64], FP32, tag="qk")
        for j in range(NCH_T):
            nn, off = j // 4, (j % 4) * 128
            g, k = nn // 4, nn % 4
            nc.tensor.matmul(saT_ps[:, j:j + 1],
                             s2[32 * k:32 * k + 16,
                                512 * g + off:512 * g + off + 128],
                             w3rep[32 * k:32 * k + 16, :],
                             start=True, stop=True, tile_position=(32 * k, 0))
        saT = mpool.tile([128, 64], FP32, tag="saT")
        nc.scalar.activation(saT[:], saT_ps[:], AF.Sigmoid)
        return saT

    def v1x1(b, xt):
        """v_pre = Wv @ x over the halo'd band -> [128, FREE] fp16"""
        vp = vppool.tile([128, FREE], FP16, tag="vp")
        c0 = 0
        while c0 < FREE:
            w = min(512, FREE - c0)
            pv = psA.tile([128, 512], FP32, tag="a")
            nc.tensor.matmul(pv[:, 0:w], wv1x1[:, :], xt[:, c0:c0 + w],
                             start=True, stop=True).annotate("mm_v1x1")
            nc.scalar.copy(vp[:, c0:c0 + w], pv[:, 0:w]).annotate("cp_vp")
            c0 += w
        return vp

    def vsum_side(b, vp):
        """vsum[c] = sum over band of v (exact, via window sums of v_pre)."""
        vv = vp[:].rearrange("p (h w) -> p h w", h=HH)

        def edge(k):
            return vv[:, :, k:k + 1].rearrange("p h w -> p (h w)")

        fr = mpool.tile([128, 34], FP32, tag="fr")
        nc.vector.tensor_reduce(fr[:], vv[:, :, :], mybir.AxisListType.X,
                                ALU.add).annotate("vsum_red")
        # rs block tj at cols [34*tj : 34*tj+34]: row sums over cols tj..tj+255
        rs = mpool.tile([128, 102], FP32, tag="rs")
        pairs = [(256, 257), (0, 257), (0, 1)]
        for tj, (ka, kb) in enumerate(pairs):
            nc.vector.tensor_tensor(rs[:, 34 * tj:34 * tj + 34], fr[:],
                                    edge(ka), ALU.subtract)
            nc.vector.tensor_tensor(rs[:, 34 * tj:34 * tj + 34],
                                    rs[:, 34 * tj:34 * tj + 34],
                                    edge(kb), ALU.subtract)
        rs3 = rs[:].rearrange("p (t r) -> p t r", t=3)
        tj_tot = mpool.tile([128, 3], FP32, tag="tjt")
        nc.vector.tensor_reduce(tj_tot[:], rs3, mybir.AxisListType.X, ALU.add)
        # ws[3*ti+tj] = tj_tot[tj] - rs[tj, ex1(ti)] - rs[tj, ex2(ti)]
        ex = [(32, 33), (0, 33), (0, 1)]
        ws = mpool.tile([128, 9], FP32, tag="ws")
        for ti in range(3):
            a_, b_ = ex[ti]
            ra = rs3[:, :, a_:a_ + 1].rearrange("p t r -> p (t r)")
            rb = rs3[:, :, b_:b_ + 1].rearrange("p t r -> p (t r)")
            nc.vector.tensor_tensor(ws[:, 3 * ti:3 * ti + 3], tj_tot[:],
                                    ra, ALU.subtract)
            nc.vector.tensor_tensor(ws[:, 3 * ti:3 * ti + 3],
                                    ws[:, 3 * ti:3 * ti + 3],
                                    rb, ALU.subtract)
        wsw = mpool.tile([128, 9], FP32, tag="wsw")
        nc.vector.tensor_tensor(wsw[:], ws[:], dwvw[:], ALU.mult)
        vsum = mpool.tile([128, 1], FP32, tag="vsum")
        nc.vector.tensor_reduce(vsum[:], wsw[:], mybir.AxisListType.X, ALU.add)
        return vsum

    def qk_gram(b, xt, saT):
        """QK 9-tap fused conv in transposed layout + gram accumulation."""
        G = psG.tile([128, 256], FP32, tag="G")
        G2 = psG.tile([128, 128], FP32, tag="G2")
        for j in range(NCH_T):
            r, c0 = j // 2, (j % 2) * 128
            pqk = psQK.tile([128, 256], FP32, tag="qk")
            for t in range(9):
                ti, tj = t // 3, t % 3
                base = (r + ti) * WW + c0 + tj
                nc.tensor.matmul(pqk[:, :], xt[:, base:base + 128],
                                 w9qk[:, 256 * t:256 * t + 256],
                                 start=(t == 0), stop=(t == 8)).annotate("mm_qk")
            rt = rpool.tile([128, 256], FP16, tag="ring")
            nc.vector.tensor_scalar_mul(rt[:, 0:128], pqk[:, 0:128],
                                        saT[:, j:j + 1]).annotate("cp_rtq")
            nc.scalar.copy(rt[:, 128:256], pqk[:, 128:256]).annotate("cp_rtk")
            nc.tensor.matmul(G[:, 0:256], rt[:, 0:128], rt[:, 0:256],
                             start=(j == 0), stop=(j == NCH_T - 1),
                             skip_group_check=True).annotate("mm_gram")
            nc.tensor.matmul(G2[:, :], rt[:, 128:256], rt[:, 128:256],
                             start=(j == 0), stop=(j == NCH_T - 1),
                             skip_group_check=True).annotate("mm_gram")
        return G, G2

    def stage_stats(b, G, G2, vsum):
        """arst [128, 131]: [Gqk | qd | kd | vsum]"""
        arst = mpool.tile([128, 131], FP32, tag=f"arst{b}")
        junk = mpool.tile([128, 128], FP32, tag="junk")
        nc.vector.tensor_copy(arst[:, 0:128], G[:, 128:256])
        nc.vector.scalar_tensor_tensor(junk[:], G[:, 0:128], 1.0, eye,
                                       ALU.mult, ALU.mult,
                                       accum_out=arst[:, 128:129])
        nc.vector.scalar_tensor_tensor(junk[:], G2[:, :], 1.0, eye,
                                       ALU.mult, ALU.mult,
                                       accum_out=arst[:, 129:130])
        nc.vector.tensor_copy(arst[:, 130:131], vsum[:])
        return arst

    def issue_ar(b, arst):
        din = dpool.tile([128, 131], FP32, tag=f"din{b}")
        dout = dpool.tile([128, 131], FP32, tag=f"dout{b}")
        dma(din[:], arst[:])
        nc.gpsimd.collective_compute(
            "AllReduce", ALU.add,
            replica_groups=[list(range(NCORES))],
            ins=[din[:].opt()], outs=[dout[:].opt()])
        arres = mpool.tile([128, 131], FP32, tag=f"arres{b}")
        dma(arres[:], dout[:])
        return arres

    def taps(spec, src, dwv, acc_pool, acc_tag):
        """depthwise 3x3 over halo'd src [128, FREE] -> acc [128, NLOC] fp16."""
        sv = src[:].rearrange("p (h w) -> p h w", h=HH)
        acc = acc_pool.tile([128, NLOC], FP16, tag=acc_tag)
        d_taps = [t for t in range(9) if spec[t] == "d"]
        g_taps = [t for t in range(9) if spec[t] == "G"]
        te_taps = [t for t in range(9) if spec[t] == "t"]

        def shifted(t, c0, w):
            # column window [c0, c0+w) of the band output, rows 0..31
            ti, tj = t // 3, t % 3
            r0, cw = c0 // W, c0 % W
            nr = w // W
            return sv[:, ti + r0:ti + r0 + nr, tj + cw:tj + cw + W]

        first = True
        for t in d_taps:
            c0 = 0
            dstv = acc[:].rearrange("p (h w) -> p h w", h=RPC)
            if first:
                nc.vector.tensor_scalar_mul(dstv, shifted(t, 0, NLOC),
                                            dwv[:, t:t + 1]).annotate("tap_ts")
            else:
                tmp = tmppool.tile([128, NLOC], FP16, tag="tmp")
                tmpv = tmp[:].rearrange("p (h w) -> p h w", h=RPC)
                nc.vector.tensor_scalar_mul(tmpv, shifted(t, 0, NLOC),
                                            dwv[:, t:t + 1]).annotate("tap_ts")
                nc.vector.tensor_tensor(acc[:], acc[:], tmp[:],
                                        ALU.add).annotate("tap_tt")
            first = False
        for t in g_taps:
            # self-contained gpsimd tap: both mul and add on GpSimd
            for h in range(NLOC // TMPG):
                c0 = h * TMPG
                tmpg = tmppool.tile([128, TMPG], FP16, tag="tmpg")
                tmpgv = tmpg[:].rearrange("p (h w) -> p h w", h=TMPG // W)
                nc.gpsimd.tensor_scalar_mul(tmpgv, shifted(t, c0, TMPG),
                                            dwv[:, t:t + 1]).annotate("tap_gts")
                nc.gpsimd.tensor_tensor(acc[:, c0:c0 + TMPG],
                                        acc[:, c0:c0 + TMPG],
                                        tmpg[:], ALU.add).annotate("tap_gtt")
        return acc, te_taps

    def taps_te(b, src, w9, te_taps, acc, merge):
        """TE dense-fold taps over halo'd src, baseline-style; merge into acc."""
        if not te_taps:
            return
        sv = src[:].rearrange("p (h w) -> p h w", h=HH)
        for nn in range(NCH_A):
            r0 = 2 * nn
            py = psA.tile([128, 512], FP32, tag="a")
            for i, t in enumerate(te_taps):
                ti, tj = t // 3, t % 3
                xv = sv[:, r0 + ti:r0 + ti + 2, tj:tj + 256]
                nc.tensor.matmul(py[:, :], w9[:, 128 * t:128 * t + 128], xv,
                                 start=(i == 0),
                                 stop=(i == len(te_taps) - 1)).annotate("mm_yte")
            if merge:
                nc.vector.tensor_tensor(acc[:, 512 * nn:512 * nn + 512],
                                        acc[:, 512 * nn:512 * nn + 512],
                                        py[:, :], ALU.add).annotate("cp_te")
            else:
                nc.scalar.copy(acc[:, 512 * nn:512 * nn + 512],
                               py[:, :]).annotate("cp_te")

    def post_ar(b, arres):
        """norms -> softmax -> Meff/p2t; returns (mefft fp16, p2t fp16)"""
        rqk = mpool.tile([128, 2], FP32, tag="rqk")
        srt = mpool.tile([128, 2], FP32, tag="srt")
        dcat = arres[:, 128:130]
        nc.scalar.activation(srt[:], dcat, AF.Sqrt)
        nc.vector.tensor_scalar_max(srt[:], srt[:], 1e-12)
        nc.vector.reciprocal(rqk[:], srt[:])
        r2 = mpool.tile([128, 2], FP32, tag="r2")
        nc.vector.tensor_tensor(r2[:], rqk[:], rqk[:], ALU.mult)
        nc.vector.tensor_tensor(r2[:], r2[:], dcat, ALU.mult)
        nc.vector.tensor_scalar(r2[:], r2[:], -0.5, 1.5, ALU.mult, ALU.add)
        nc.vector.tensor_tensor(rqk[:], rqk[:], r2[:], ALU.mult)
        rqt = mpool.tile([128, 1], FP32, tag="rqt")
        nc.vector.tensor_tensor(rqt[:], rqk[:, 0:1], tempp, ALU.mult)

        ps1 = psA.tile([128, 128], FP32, tag="a")
        nc.tensor.matmul(ps1[0:1, :], rqk[:, 1:2], eye, start=True, stop=True)
        rkrow = mpool.tile([1, 128], FP32, tag="rkrow")
        nc.scalar.copy(rkrow[:], ps1[0:1, :])
        ps2 = psA.tile([128, 128], FP32, tag="a")
        nc.tensor.matmul(ps2[:, :], onesrow, rkrow[:], start=True, stop=True)

        gh = mpool.tile([128, 128], FP32, tag="gh")
        nc.vector.scalar_tensor_tensor(gh[:], arres[:, 0:128], rqt[:, 0:1],
                                       ps2[:, :], ALU.mult, ALU.mult)
        sm = mpool.tile([128, 128], FP32, tag="sm")
        nc.scalar.activation(sm[:], gh[:], AF.Exp)
        rs_ = mpool.tile([128, 1], FP32, tag="rssm")
        nc.vector.scalar_tensor_tensor(sm[:], sm[:], 1.0, bdmask,
                                       ALU.mult, ALU.mult, accum_out=rs_[:])
        nc.vector.reciprocal(rs_[:], rs_[:])
        attn = mpool.tile([128, 128], FP32, tag="attn")
        nc.vector.tensor_scalar_mul(attn[:], sm[:], rs_[:, 0:1])

        psM = psA.tile([128, 128], FP32, tag="a")
        nc.tensor.matmul(psM[:, :], attn[:], projt[:], start=True, stop=True)
        mefft = mpool.tile([128, 128], FP16, tag="mefft")
        nc.scalar.copy(mefft[:], psM[:, :])

        psT = psA.tile([128, 128], FP32, tag="a")
        nc.tensor.transpose(psT[:, :], attn[:], eye)
        attnt = mpool.tile([128, 128], FP32, tag="attnt")
        nc.vector.tensor_copy(attnt[:], psT[:, :])
        psP = psA.tile([128, 1], FP32, tag="a")
        nc.tensor.matmul(psP[:, :], attnt[:], arres[:, 130:131],
                         start=True, stop=True)
        pooled = mpool.tile([128, 1], FP32, tag="pooled")
        nc.scalar.activation(pooled[:], psP[:, :], AF.Copy, scale=1.0 / NTOT)

        psg1 = psA.tile([16, 1], FP32, tag="a")
        nc.tensor.matmul(psg1[:, :], spw1t[:], pooled[:], start=True, stop=True)
        g1 = mpool.tile([16, 1], FP32, tag="g1")
        nc.scalar.activation(g1[:], psg1[:, :], AF.Gelu)
        psg2 = psA.tile([16, 1], FP32, tag="a")
        nc.tensor.matmul(psg2[:, :], spw2t[:], g1[:], start=True, stop=True)
        g2 = mpool.tile([16, 1], FP32, tag="g2")
        nc.scalar.activation(g2[:], psg2[:, :], AF.Gelu)
        psg3 = psA.tile([128, 1], FP32, tag="a")
        nc.tensor.matmul(psg3[:, :], spw3t[:], g2[:], start=True, stop=True)
        spec = mpool.tile([128, 1], FP32, tag="spec")
        nc.scalar.activation(spec[:], psg3[:, :], AF.Sigmoid)

        p2t = mpool.tile([128, 128], FP16, tag="p2t")
        nc.vector.tensor_scalar_mul(p2t[:], projt[:], spec[:, 0:1])
        return mefft, p2t

    def final_proj(b, mefft, p2t, vt, y2t):
        out2d = io["out"][b].rearrange("c h w -> c (h w)")
        for nn in range(NCH_A):
            pf = psA.tile([128, 512], FP32, tag="a")
            nc.tensor.matmul(pf[:, :], mefft[:],
                             vt[:, 512 * nn:512 * nn + 512],
                             start=True, stop=False).annotate("mm_proj")
            nc.tensor.matmul(pf[:, :], p2t[:],
                             y2t[:, 512 * nn:512 * nn + 512],
                             start=False, stop=True).annotate("mm_proj")
            ot = opool.tile([128, 512], FP16, tag="ot")
            nc.scalar.copy(ot[:], pf[:, :]).annotate("cp_ot")
            dma(out2d[:, 512 * nn:512 * nn + 512], ot[:])

    # ================= schedule =================
    with nc.allow_low_precision(reason="fp16 depthwise accumulation"):
        for b in range(B):
            xt = xpool.tile([128, FREE], BF16, tag="x")
            yt = ypool.tile([128, FREE], BF16, tag="y")
            dma(yt[:], io["yh"][b].rearrange("c h w -> c (h w)"))
            dma(xt[:], io["xh"][b].rearrange("c h w -> c (h w)"))
            xts.append(xt)
            yts.append(yt)

            saT = sa_gate(b, yt)
            saTs.append(saT)
            vp = v1x1(b, xt)
            vsum = vsum_side(b, vp)
            G, G2 = qk_gram(b, xt, saT)
            arst = stage_stats(b, G, G2, vsum)
            arres = issue_ar(b, arst)
            arreses.append(arres)

            # DVE/GP taps (no dep on the AllReduce; fill its window)
            vt, v_te = taps(V_TAPS[b], vp, dwvw, vpool, "vt")
            taps_te(b, xt, w9v, v_te, vt, merge=True)
            vts.append(vt)
            y_spec = Y_TAPS[b]
            if all(s == "t" for s in y_spec):
                y2t = y2pool.tile([128, NLOC], FP16, tag="y2t")
                taps_te(b, yt, wyd, list(range(9)), y2t, merge=False)
            else:
                y2t, y_te = taps(y_spec, yt, dwyw, y2pool, "y2t")
                taps_te(b, yt, wyd, y_te, y2t, merge=True)
            y2ts.append(y2t)

        for b in range(B):
            mefft, p2t = post_ar(b, arreses[b])
            final_proj(b, mefft, p2t, vts[b], y2ts[b])

    ctx.close()


def build_nc():
    nc = bacc.Bacc("TRN2", target_bir_lowering=False, debug=False,
                   num_devices=NCORES)
    io = {}

    def inp(name, shape, dt):
        io[name] = nc.dram_tensor(name, shape, dt, kind="ExternalInput")

    inp("xh", [B, C, HH, WW], BF16)
    inp("yh", [B, C, HH, WW], BF16)
    inp("w9qk", [128, 9 * 256], BF16)
    inp("w9v", [128, 9 * 128], BF16)
    inp("wyd", [128, 9 * 128], BF16)
    inp("wv1x1", [128, 128], BF16)
    inp("saw1t", [128, 32], BF16)
    inp("w2rep", [128, 32], BF16)
    inp("w3rep", [128, 1], BF16)
    inp("spw1t", [128, 16], FP32)
    inp("spw2t", [16, 16], FP32)
    inp("spw3t", [16, 128], FP32)
    inp("projt", [128, 128], FP32)
    inp("dwvw", [128, 9], FP32)
    inp("dwyw", [128, 9], FP32)
    inp("consts", [128, 386], FP32)
    io["out"] = nc.dram_tensor("out", [B, C, RPC, W], FP16, kind="ExternalOutput")

    with tile.TileContext(nc) as tc:
        _emit(tc, io)
    nc.finalize()
    return nc


_CACHE = {}


def _prep_host(x, y, qkv_w, qkv_dw_w, proj_w, sa_w1, sa_w2, sa_w3,
               sp_w1, sp_w2, sp_w3, dw_w, temperature):
    import ml_dtypes
    bf = ml_dtypes.bfloat16
    f32 = np.float32

    x = np.asarray(x, f32)
    y = np.asarray(y, f32)
    xp = np.zeros((B, C, H + 2, W + 2), f32)
    xp[:, :, 1:H + 1, 1:W + 1] = x
    yp = np.zeros((B, C, H + 2, W + 2), f32)
    yp[:, :, 1:H + 1, 1:W + 1] = y
    xp = xp.astype(bf)
    yp = yp.astype(bf)

    qkv_w = np.asarray(qkv_w, f32)
    dw = np.asarray(qkv_dw_w, f32).reshape(3 * C, 9)
    w9qk = np.concatenate(
        [(qkv_w[:256] * dw[:256, t:t + 1]).T for t in range(9)], axis=1)
    w9v = np.concatenate(
        [(qkv_w[256:] * dw[256:, t:t + 1]).T for t in range(9)], axis=1)
    dwy = np.asarray(dw_w, f32).reshape(C, 9)
    wyd = np.concatenate(
        [np.diag(dwy[:, t]) for t in range(9)], axis=1)

    w2rep = np.zeros((128, 32), f32)
    w3rep = np.zeros((128, 1), f32)
    for k in range(4):
        w2rep[32 * k:32 * k + 16, 0:16] = np.asarray(sa_w2, f32).T
        w3rep[32 * k:32 * k + 16] = np.asarray(sa_w3, f32).T
    saw1tp = np.zeros((128, 32), f32)
    saw1tp[:, 0:16] = np.asarray(sa_w1, f32).T

    consts = np.zeros((128, 386), f32)
    consts[:, 0:128] = np.eye(128, dtype=f32)
    ci = np.arange(128) // DH
    consts[:, 128:256] = (ci[:, None] == ci[None, :]).astype(f32)
    consts[:, 256] = np.asarray(temperature, f32).reshape(HD)[ci]
    consts[0, 257:385] = 1.0

    common = {
        "w9qk": w9qk.astype(bf), "w9v": w9v.astype(bf), "wyd": wyd.astype(bf),
        "wv1x1": np.ascontiguousarray(qkv_w[256:].T).astype(bf),
        "saw1t": saw1tp.astype(bf),
        "w2rep": w2rep.astype(bf), "w3rep": w3rep.astype(bf),
        "spw1t": np.asarray(sp_w1, f32).T.copy(),
        "spw2t": np.asarray(sp_w2, f32).T.copy(),
        "spw3t": np.asarray(sp_w3, f32).T.copy(),
        "projt": np.asarray(proj_w, f32).T.copy(),
        "dwvw": np.ascontiguousarray(dw[256:]),
        "dwyw": np.ascontiguousarray(dwy),
        "consts": consts,
    }
    in_maps = []
    for i in range(NCORES):
        m = dict(common)
        m["xh"] = np.ascontiguousarray(xp[:, :, 32 * i:32 * i + HH, :])
        m["yh"] = np.ascontiguousarray(yp[:, :, 32 * i:32 * i + HH, :])
        in_maps.append(m)
    return in_maps


def kernel(**inputs):
    if "nc" not in _CACHE:
        _CACHE["nc"] = build_nc()
    nc = _CACHE["nc"]
    in_maps = _prep_host(**inputs)
    res = run_bass_kernel_spmd(nc, in_maps, core_ids=list(range(NCORES)))
    shards = [res.results[i]["out"] for i in range(NCORES)]
    return np.concatenate(shards, axis=2).astype(np.float32)


# revision 8
# speedup vs baseline: 2.5973x; 2.5973x over previous
"""Cross-Spatial-Attention Trainium2 kernel (8 NeuronCores, spatial sharding).

v2: engine-balanced. TensorE keeps the QK 9-tap fused conv (transposed
layout) + gram + SA gate + projections; the depthwise work for v and
dwconv(y) moves to the Vector/GpSimd engines as per-channel
multiply-accumulate passes (tensor_scalar 4x + tensor_tensor 2x, fp16),
fed by a cheap 1x1 conv for v_pre. The v-mean needed by the stats
AllReduce is computed from window sums of v_pre (row-sum side path) so
each batch's AllReduce fires right after its QK gram; batch1's
y-depthwise stays on TensorE as filler inside the AllReduce window.
"""

import numpy as np
from contextlib import ExitStack

import concourse.bass as bass
import concourse.bacc as bacc
import concourse.tile as tile
from concourse import mybir
from concourse.bass_utils import run_bass_kernel_spmd

FP32 = mybir.dt.float32
FP16 = mybir.dt.float16
BF16 = mybir.dt.bfloat16
AF = mybir.ActivationFunctionType
ALU = mybir.AluOpType

B, C, H, W = 2, 128, 256, 256
HD, DH = 8, 16
NCORES = 8
RPC = H // NCORES            # 32 rows per core
HH, WW = RPC + 2, W + 2      # 34 x 258 halo'd band
FREE = HH * WW               # 8772
NLOC = RPC * W               # 8192 output positions per band per batch
NCH_T = NLOC // 128          # 64 transposed chunks
NCH_A = NLOC // 512          # 16 layout-A chunks
NTOT = float(H * W)          # global spatial size

# tap engine assignment: per (tensor, batch) a list of 9 entries
# 'd' = DVE ts+tt, 'g' = DVE ts + GpSimd tt, 't' = TensorE dense fold
V_TAPS = {0: list("ddddddddd"), 1: list("ddddddddd")}
Y_TAPS = {0: list("ddddddddd"), 1: list("ttttttttt")}

TMPH = NLOC                  # DVE tap chunk (full width)
TMPG = NLOC // 2             # gpsimd tap chunk


def _emit(tc, io):
    nc = tc.nc
    ctx = ExitStack()

    wpool = ctx.enter_context(tc.tile_pool(name="wpool", bufs=1))
    xpool = ctx.enter_context(tc.tile_pool(name="xpool", bufs=1))
    ypool = ctx.enter_context(tc.tile_pool(name="ypool", bufs=2))
    vppool = ctx.enter_context(tc.tile_pool(name="vppool", bufs=2))
    vpool = ctx.enter_context(tc.tile_pool(name="vpool", bufs=2))
    y2pool = ctx.enter_context(tc.tile_pool(name="y2pool", bufs=2))
    tmppool = ctx.enter_context(tc.tile_pool(name="tmppool", bufs=1))
    spool = ctx.enter_context(tc.tile_pool(name="spool", bufs=1))
    rpool = ctx.enter_context(tc.tile_pool(name="rpool", bufs=4))
    mpool = ctx.enter_context(tc.tile_pool(name="mpool", bufs=1))
    opool = ctx.enter_context(tc.tile_pool(name="opool", bufs=2))
    psA = ctx.enter_context(tc.tile_pool(name="psA", bufs=2, space="PSUM"))
    psQK = ctx.enter_context(tc.tile_pool(name="psQK", bufs=3, space="PSUM"))
    psG = ctx.enter_context(tc.tile_pool(name="psG", bufs=1, space="PSUM"))
    dpool = ctx.enter_context(tc.tile_pool(name="dram", bufs=4, space="DRAM"))

    def dma(dst, src):
        nc.sync.dma_start(out=dst, in_=src)

    def wload(name, shape, dt=BF16):
        t = wpool.tile(shape, dt, tag=name)
        dma(t[:], io[name][:])
        return t

    w9qk = wload("w9qk", [128, 9 * 256])     # tap t at cols [256t:256t+256]
    need_w9v = any(s == "t" for b in range(B) for s in V_TAPS[b])
    w9v = wload("w9v", [128, 9 * 128]) if need_w9v else None
    wyd = wload("wyd", [128, 9 * 128])       # diag taps for TE-assigned y taps
    wv1x1 = wload("wv1x1", [128, 128])       # v 1x1: [ic, oc]
    saw1t = wload("saw1t", [128, 32])
    w2rep = wload("w2rep", [128, 32])
    w3rep = wload("w3rep", [128, 1])
    spw1t = wload("spw1t", [128, 16], FP32)
    spw2t = wload("spw2t", [16, 16], FP32)
    spw3t = wload("spw3t", [16, 128], FP32)
    projt = wload("projt", [128, 128], FP32)
    dwvw = wload("dwvw", [128, 9], FP32)     # v depthwise tap weights
    dwyw = wload("dwyw", [128, 9], FP32)     # y depthwise tap weights
    consts = wload("consts", [128, 386], FP32)
    eye = consts[:, 0:128]
    bdmask = consts[:, 128:256]
    tempp = consts[:, 256:257]
    onesrow = consts[0:1, 257:385]

    xts, yts, vts, y2ts, saTs = [], [], [], [], []
    arreses = []

    # ---------------- helpers ----------------
    def sa_gate(b, yt):
        """spatial-attention gate -> saT [128, 64] (col j = chunk j)"""
        s1 = spool.tile([128, 2048], BF16, tag="s1")
        s2 = s1
        for g in range(4):
            ps1 = psA.tile([128, 512], FP32, tag="a")
            for k in range(4):
                nn = 4 * g + k
                r0 = 2 * nn
                yv = yt[:].rearrange("p (h w) -> p h w", h=HH)[
                    :, r0 + 1:r0 + 3, 1:257]
                nc.tensor.matmul(ps1[32 * k:32 * k + 32, :], saw1t[:, :], yv,
                                 start=True, stop=True,
                                 tile_position=(0, 32 * k)).annotate("mm_sa")
            if g % 2 == 0:
                nc.vector.tensor_scalar_max(s1[:, 512 * g:512 * g + 512], ps1[:, :], 0.0)
            else:
                nc.scalar.activation(s1[:, 512 * g:512 * g + 512], ps1[:, :], AF.Relu)
        for g in range(4):
            ps2 = psA.tile([128, 512], FP32, tag="a")
            for k in range(4):
                nc.tensor.matmul(ps2[32 * k:32 * k + 32, :],
                                 w2rep[32 * k:32 * k + 16, :],
                                 s1[32 * k:32 * k + 16, 512 * g:512 * g + 512],
                                 start=True, stop=True,
                                 tile_position=(32 * k, 32 * k))
            if g % 2 == 0:
                nc.vector.tensor_scalar_max(s2[:, 512 * g:512 * g + 512], ps2[:, :], 0.0)
            else:
                nc.scalar.activation(s2[:, 512 * g:512 * g + 512], ps2[:, :], AF.Relu)
        saT_ps = psQK.tile([128, 64], FP32, tag="qk")
        for j in range(NCH_T):
            nn, off = j // 4, (j % 4) * 128
            g, k = nn // 4, nn % 4
            nc.tensor.matmul(saT_ps[:, j:j + 1],
                             s2[32 * k:32 * k + 16,
                                512 * g + off:512 * g + off + 128],
                             w3rep[32 * k:32 * k + 16, :],
                             start=True, stop=True, tile_position=(32 * k, 0))
        saT = mpool.tile([128, 64], FP32, tag="saT")
        nc.scalar.activation(saT[:], saT_ps[:], AF.Sigmoid)
        return saT

    def v1x1(b, xt):
        """v_pre = Wv @ x over the halo'd band -> [128, FREE] fp16"""
        vp = vppool.tile([128, FREE], FP16, tag="vp")
        c0 = 0
        while c0 < FREE:
            w = min(512, FREE - c0)
            pv = psA.tile([128, 512], FP32, tag="a")
            nc.tensor.matmul(pv[:, 0:w], wv1x1[:, :], xt[:, c0:c0 + w],
                             start=True, stop=True).annotate("mm_v1x1")
            nc.scalar.copy(vp[:, c0:c0 + w], pv[:, 0:w]).annotate("cp_vp")
            c0 += w
        return vp

    def vsum_side(b, vp):
        """vsum[c] = sum over band of v (exact, via window sums of v_pre)."""
        vv = vp[:].rearrange("p (h w) -> p h w", h=HH)

        def edge(k):
            return vv[:, :, k:k + 1].rearrange("p h w -> p (h w)")

        fr = mpool.tile([128, 34], FP16, tag="fr")
        nc.vector.tensor_reduce(fr[:], vv[:, :, :], mybir.AxisListType.X,
                                ALU.add).annotate("vsum_red")
        # rs block tj at cols [34*tj : 34*tj+34]: row sums over cols tj..tj+255
        rs = mpool.tile([128, 102], FP32, tag="rs")
        pairs = [(256, 257), (0, 257), (0, 1)]
        for tj, (ka, kb) in enumerate(pairs):
            nc.vector.tensor_tensor(rs[:, 34 * tj:34 * tj + 34], fr[:],
                                    edge(ka), ALU.subtract)
            nc.vector.tensor_tensor(rs[:, 34 * tj:34 * tj + 34],
                                    rs[:, 34 * tj:34 * tj + 34],
                                    edge(kb), ALU.subtract)
        rs3 = rs[:].rearrange("p (t r) -> p t r", t=3)
        tj_tot = mpool.tile([128, 3], FP32, tag="tjt")
        nc.vector.tensor_reduce(tj_tot[:], rs3, mybir.AxisListType.X, ALU.add)
        # ws[3*ti+tj] = tj_tot[tj] - rs[tj, ex1(ti)] - rs[tj, ex2(ti)]
        ex = [(32, 33), (0, 33), (0, 1)]
        ws = mpool.tile([128, 9], FP32, tag="ws")
        for ti in range(3):
            a_, b_ = ex[ti]
            ra = rs3[:, :, a_:a_ + 1].rearrange("p t r -> p (t r)")
            rb = rs3[:, :, b_:b_ + 1].rearrange("p t r -> p (t r)")
            nc.vector.tensor_tensor(ws[:, 3 * ti:3 * ti + 3], tj_tot[:],
                                    ra, ALU.subtract)
            nc.vector.tensor_tensor(ws[:, 3 * ti:3 * ti + 3],
                                    ws[:, 3 * ti:3 * ti + 3],
                                    rb, ALU.subtract)
        wsw = mpool.tile([128, 9], FP32, tag="wsw")
        nc.vector.tensor_tensor(wsw[:], ws[:], dwvw[:], ALU.mult)
        vsum = mpool.tile([128, 1], FP32, tag="vsum")
        nc.vector.tensor_reduce(vsum[:], wsw[:], mybir.AxisListType.X, ALU.add)
        return vsum

    def qk_gram(b, xt, saT):
        """QK 9-tap fused conv in transposed layout + gram accumulation."""
        G = psG.tile([128, 256], FP32, tag="G")
        G2 = psG.tile([128, 128], FP32, tag="G2")
        for j in range(NCH_T):
            r, c0 = j // 2, (j % 2) * 128
            pqk = psQK.tile([128, 256], FP32, tag="qk")
            for t in range(9):
                ti, tj = t // 3, t % 3
                base = (r + ti) * WW + c0 + tj
                nc.tensor.matmul(pqk[:, :], xt[:, base:base + 128],
                                 w9qk[:, 256 * t:256 * t + 256],
                                 start=(t == 0), stop=(t == 8)).annotate("mm_qk")
            rt = rpool.tile([128, 256], BF16, tag="ring")
            nc.scalar.activation(rt[:, 0:128], pqk[:, 0:128], AF.Copy,
                                 scale=saT[:, j:j + 1]).annotate("cp_rtq")
            nc.vector.tensor_copy(rt[:, 128:256], pqk[:, 128:256]).annotate("cp_rtk")
            nc.tensor.matmul(G[:, 0:256], rt[:, 0:128], rt[:, 0:256],
                             start=(j == 0), stop=(j == NCH_T - 1),
                             skip_group_check=True).annotate("mm_gram")
            nc.tensor.matmul(G2[:, :], rt[:, 128:256], rt[:, 128:256],
                             start=(j == 0), stop=(j == NCH_T - 1),
                             skip_group_check=True).annotate("mm_gram")
        return G, G2

    def stage_stats(b, G, G2, vsum):
        """arst [128, 131]: [Gqk | qd | kd | vsum]"""
        arst = mpool.tile([128, 131], FP32, tag=f"arst{b}")
        junk = mpool.tile([128, 128], FP32, tag="junk")
        nc.vector.tensor_copy(arst[:, 0:128], G[:, 128:256])
        nc.vector.scalar_tensor_tensor(junk[:], G[:, 0:128], 1.0, eye,
                                       ALU.mult, ALU.mult,
                                       accum_out=arst[:, 128:129])
        nc.vector.scalar_tensor_tensor(junk[:], G2[:, :], 1.0, eye,
                                       ALU.mult, ALU.mult,
                                       accum_out=arst[:, 129:130])
        nc.vector.tensor_copy(arst[:, 130:131], vsum[:])
        return arst

    def issue_ar(b, arst):
        din = dpool.tile([128, 131], FP32, tag=f"din{b}")
        dout = dpool.tile([128, 131], FP32, tag=f"dout{b}")
        dma(din[:], arst[:])
        nc.gpsimd.collective_compute(
            "AllReduce", ALU.add,
            replica_groups=[list(range(NCORES))],
            ins=[din[:].opt()], outs=[dout[:].opt()])
        arres = mpool.tile([128, 131], FP32, tag=f"arres{b}")
        dma(arres[:], dout[:])
        return arres

    def taps(spec, src, dwv, acc_pool, acc_tag):
        """depthwise 3x3 over halo'd src [128, FREE] -> acc [128, NLOC] fp16."""
        sv = src[:].rearrange("p (h w) -> p h w", h=HH)
        acc = acc_pool.tile([128, NLOC], FP16, tag=acc_tag)
        d_taps = [t for t in range(9) if spec[t] == "d"]
        g_taps = [t for t in range(9) if spec[t] == "G"]
        te_taps = [t for t in range(9) if spec[t] == "t"]

        def shifted(t, c0, w):
            # column window [c0, c0+w) of the band output, rows 0..31
            ti, tj = t // 3, t % 3
            r0, cw = c0 // W, c0 % W
            nr = w // W
            return sv[:, ti + r0:ti + r0 + nr, tj + cw:tj + cw + W]

        first = True
        for t in d_taps:
            c0 = 0
            dstv = acc[:].rearrange("p (h w) -> p h w", h=RPC)
            if first:
                nc.vector.tensor_scalar_mul(dstv, shifted(t, 0, NLOC),
                                            dwv[:, t:t + 1]).annotate("tap_ts")
            else:
                tmp = tmppool.tile([128, NLOC], FP16, tag="tmp")
                tmpv = tmp[:].rearrange("p (h w) -> p h w", h=RPC)
                nc.vector.tensor_scalar_mul(tmpv, shifted(t, 0, NLOC),
                                            dwv[:, t:t + 1]).annotate("tap_ts")
                nc.vector.tensor_tensor(acc[:], acc[:], tmp[:],
                                        ALU.add).annotate("tap_tt")
            first = False
        for t in g_taps:
            # self-contained gpsimd tap: both mul and add on GpSimd
            for h in range(NLOC // TMPG):
                c0 = h * TMPG
                tmpg = tmppool.tile([128, TMPG], FP16, tag="tmpg")
                tmpgv = tmpg[:].rearrange("p (h w) -> p h w", h=TMPG // W)
                nc.gpsimd.tensor_scalar_mul(tmpgv, shifted(t, c0, TMPG),
                                            dwv[:, t:t + 1]).annotate("tap_gts")
                nc.gpsimd.tensor_tensor(acc[:, c0:c0 + TMPG],
                                        acc[:, c0:c0 + TMPG],
                                        tmpg[:], ALU.add).annotate("tap_gtt")
        return acc, te_taps

    def taps_te(b, src, w9, te_taps, acc, merge):
        """TE dense-fold taps over halo'd src, baseline-style; merge into acc."""
        if not te_taps:
            return
        sv = src[:].rearrange("p (h w) -> p h w", h=HH)
        for nn in range(NCH_A):
            r0 = 2 * nn
            py = psA.tile([128, 512], FP32, tag="a")
            for i, t in enumerate(te_taps):
                ti, tj = t // 3, t % 3
                xv = sv[:, r0 + ti:r0 + ti + 2, tj:tj + 256]
                nc.tensor.matmul(py[:, :], w9[:, 128 * t:128 * t + 128], xv,
                                 start=(i == 0),
                                 stop=(i == len(te_taps) - 1)).annotate("mm_yte")
            if merge:
                nc.vector.tensor_tensor(acc[:, 512 * nn:512 * nn + 512],
                                        acc[:, 512 * nn:512 * nn + 512],
                                        py[:, :], ALU.add).annotate("cp_te")
            else:
                nc.scalar.copy(acc[:, 512 * nn:512 * nn + 512],
                               py[:, :]).annotate("cp_te")

    def post_ar(b, arres):
        """norms -> softmax -> Meff/p2t; returns (mefft fp16, p2t fp16)"""
        rqk = mpool.tile([128, 2], FP32, tag="rqk")
        srt = mpool.tile([128, 2], FP32, tag="srt")
        dcat = arres[:, 128:130]
        nc.scalar.activation(srt[:], dcat, AF.Sqrt)
        nc.vector.tensor_scalar_max(srt[:], srt[:], 1e-12)
        nc.vector.reciprocal(rqk[:], srt[:])
        r2 = mpool.tile([128, 2], FP32, tag="r2")
        nc.vector.tensor_tensor(r2[:], rqk[:], rqk[:], ALU.mult)
        nc.vector.tensor_tensor(r2[:], r2[:], dcat, ALU.mult)
        nc.vector.tensor_scalar(r2[:], r2[:], -0.5, 1.5, ALU.mult, ALU.add)
        nc.vector.tensor_tensor(rqk[:], rqk[:], r2[:], ALU.mult)
        rqt = mpool.tile([128, 1], FP32, tag="rqt")
        nc.vector.tensor_tensor(rqt[:], rqk[:, 0:1], tempp, ALU.mult)

        ps1 = psA.tile([128, 128], FP32, tag="a")
        nc.tensor.matmul(ps1[0:1, :], rqk[:, 1:2], eye, start=True, stop=True)
        rkrow = mpool.tile([1, 128], FP32, tag="rkrow")
        nc.scalar.copy(rkrow[:], ps1[0:1, :])
        ps2 = psA.tile([128, 128], FP32, tag="a")
        nc.tensor.matmul(ps2[:, :], onesrow, rkrow[:], start=True, stop=True)

        gh = mpool.tile([128, 128], FP32, tag="gh")
        nc.vector.scalar_tensor_tensor(gh[:], arres[:, 0:128], rqt[:, 0:1],
                                       ps2[:, :], ALU.mult, ALU.mult)
        sm = mpool.tile([128, 128], FP32, tag="sm")
        nc.scalar.activation(sm[:], gh[:], AF.Exp)
        rs_ = mpool.tile([128, 1], FP32, tag="rssm")
        nc.vector.scalar_tensor_tensor(sm[:], sm[:], 1.0, bdmask,
                                       ALU.mult, ALU.mult, accum_out=rs_[:])
        nc.vector.reciprocal(rs_[:], rs_[:])
        attn = mpool.tile([128, 128], FP32, tag="attn")
        nc.vector.tensor_scalar_mul(attn[:], sm[:], rs_[:, 0:1])

        psM = psA.tile([128, 128], FP32, tag="a")
        nc.tensor.matmul(psM[:, :], attn[:], projt[:], start=True, stop=True)
        mefft = mpool.tile([128, 128], FP16, tag="mefft")
        nc.scalar.copy(mefft[:], psM[:, :])

        psT = psA.tile([128, 128], FP32, tag="a")
        nc.tensor.transpose(psT[:, :], attn[:], eye)
        attnt = mpool.tile([128, 128], FP32, tag="attnt")
        nc.vector.tensor_copy(attnt[:], psT[:, :])
        psP = psA.tile([128, 1], FP32, tag="a")
        nc.tensor.matmul(psP[:, :], attnt[:], arres[:, 130:131],
                         start=True, stop=True)
        pooled = mpool.tile([128, 1], FP32, tag="pooled")
        nc.scalar.activation(pooled[:], psP[:, :], AF.Copy, scale=1.0 / NTOT)

        psg1 = psA.tile([16, 1], FP32, tag="a")
        nc.tensor.matmul(psg1[:, :], spw1t[:], pooled[:], start=True, stop=True)
        g1 = mpool.tile([16, 1], FP32, tag="g1")
        nc.scalar.activation(g1[:], psg1[:, :], AF.Gelu)
        psg2 = psA.tile([16, 1], FP32, tag="a")
        nc.tensor.matmul(psg2[:, :], spw2t[:], g1[:], start=True, stop=True)
        g2 = mpool.tile([16, 1], FP32, tag="g2")
        nc.scalar.activation(g2[:], psg2[:, :], AF.Gelu)
        psg3 = psA.tile([128, 1], FP32, tag="a")
        nc.tensor.matmul(psg3[:, :], spw3t[:], g2[:], start=True, stop=True)
        spec = mpool.tile([128, 1], FP32, tag="spec")
        nc.scalar.activation(spec[:], psg3[:, :], AF.Sigmoid)

        p2t = mpool.tile([128, 128], FP16, tag="p2t")
        nc.vector.tensor_scalar_mul(p2t[:], projt[:], spec[:, 0:1])
        return mefft, p2t

    def final_proj(b, mefft, p2t, vt, y2t):
        out2d = io["out"][b].rearrange("c h w -> c (h w)")
        for nn in range(NCH_A):
            pf = psA.tile([128, 512], FP32, tag="a")
            nc.tensor.matmul(pf[:, :], mefft[:],
                             vt[:, 512 * nn:512 * nn + 512],
                             start=True, stop=False).annotate("mm_proj")
            nc.tensor.matmul(pf[:, :], p2t[:],
                             y2t[:, 512 * nn:512 * nn + 512],
                             start=False, stop=True).annotate("mm_proj")
            ot = opool.tile([128, 512], FP16, tag="ot")
            nc.scalar.copy(ot[:], pf[:, :]).annotate("cp_ot")
            dma(out2d[:, 512 * nn:512 * nn + 512], ot[:])

    # ================= schedule =================
    with nc.allow_low_precision(reason="fp16 depthwise accumulation"):
        for b in range(B):
            xt = xpool.tile([128, FREE], BF16, tag="x")
            yt = ypool.tile([128, FREE], BF16, tag="y")
            dma(yt[:], io["yh"][b].rearrange("c h w -> c (h w)"))
            dma(xt[:], io["xh"][b].rearrange("c h w -> c (h w)"))
            xts.append(xt)
            yts.append(yt)

            saT = sa_gate(b, yt)
            saTs.append(saT)
            vp = v1x1(b, xt)
            vsum = vsum_side(b, vp)
            G, G2 = qk_gram(b, xt, saT)
            arst = stage_stats(b, G, G2, vsum)
            arres = issue_ar(b, arst)
            arreses.append(arres)

            # DVE/GP taps (no dep on the AllReduce; fill its window)
            vt, v_te = taps(V_TAPS[b], vp, dwvw, vpool, "vt")
            taps_te(b, xt, w9v, v_te, vt, merge=True)
            vts.append(vt)
            y_spec = Y_TAPS[b]
            if all(s == "t" for s in y_spec):
                y2t = y2pool.tile([128, NLOC], FP16, tag="y2t")
                taps_te(b, yt, wyd, list(range(9)), y2t, merge=False)
            else:
                y2t, y_te = taps(y_spec, yt, dwyw, y2pool, "y2t")
                taps_te(b, yt, wyd, y_te, y2t, merge=True)
            y2ts.append(y2t)

        for b in range(B):
            mefft, p2t = post_ar(b, arreses[b])
            final_proj(b, mefft, p2t, vts[b], y2ts[b])

    ctx.close()


def build_nc():
    nc = bacc.Bacc("TRN2", target_bir_lowering=False, debug=False,
                   num_devices=NCORES)
    io = {}

    def inp(name, shape, dt):
        io[name] = nc.dram_tensor(name, shape, dt, kind="ExternalInput")

    inp("xh", [B, C, HH, WW], BF16)
    inp("yh", [B, C, HH, WW], BF16)
    inp("w9qk", [128, 9 * 256], BF16)
    inp("w9v", [128, 9 * 128], BF16)
    inp("wyd", [128, 9 * 128], BF16)
    inp("wv1x1", [128, 128], BF16)
    inp("saw1t", [128, 32], BF16)
    inp("w2rep", [128, 32], BF16)
    inp("w3rep", [128, 1], BF16)
    inp("spw1t", [128, 16], FP32)
    inp("spw2t", [16, 16], FP32)
    inp("spw3t", [16, 128], FP32)
    inp("projt", [128, 128], FP32)
    inp("dwvw", [128, 9], FP32)
    inp("dwyw", [128, 9], FP32)
    inp("consts", [128, 386], FP32)
    io["out"] = nc.dram_tensor("out", [B, C, RPC, W], FP16, kind="ExternalOutput")

    with tile.TileContext(nc) as tc:
        _emit(tc, io)
    nc.finalize()
    return nc


_CACHE = {}


def _prep_host(x, y, qkv_w, qkv_dw_w, proj_w, sa_w1, sa_w2, sa_w3,
               sp_w1, sp_w2, sp_w3, dw_w, temperature):
    import ml_dtypes
    bf = ml_dtypes.bfloat16
    f32 = np.float32

    x = np.asarray(x, f32)
    y = np.asarray(y, f32)
    xp = np.zeros((B, C, H + 2, W + 2), f32)
    xp[:, :, 1:H + 1, 1:W + 1] = x
    yp = np.zeros((B, C, H + 2, W + 2), f32)
    yp[:, :, 1:H + 1, 1:W + 1] = y
    xp = xp.astype(bf)
    yp = yp.astype(bf)

    qkv_w = np.asarray(qkv_w, f32)
    dw = np.asarray(qkv_dw_w, f32).reshape(3 * C, 9)
    w9qk = np.concatenate(
        [(qkv_w[:256] * dw[:256, t:t + 1]).T for t in range(9)], axis=1)
    w9v = np.concatenate(
        [(qkv_w[256:] * dw[256:, t:t + 1]).T for t in range(9)], axis=1)
    dwy = np.asarray(dw_w, f32).reshape(C, 9)
    wyd = np.concatenate(
        [np.diag(dwy[:, t]) for t in range(9)], axis=1)

    w2rep = np.zeros((128, 32), f32)
    w3rep = np.zeros((128, 1), f32)
    for k in range(4):
        w2rep[32 * k:32 * k + 16, 0:16] = np.asarray(sa_w2, f32).T
        w3rep[32 * k:32 * k + 16] = np.asarray(sa_w3, f32).T
    saw1tp = np.zeros((128, 32), f32)
    saw1tp[:, 0:16] = np.asarray(sa_w1, f32).T

    consts = np.zeros((128, 386), f32)
    consts[:, 0:128] = np.eye(128, dtype=f32)
    ci = np.arange(128) // DH
    consts[:, 128:256] = (ci[:, None] == ci[None, :]).astype(f32)
    consts[:, 256] = np.asarray(temperature, f32).reshape(HD)[ci]
    consts[0, 257:385] = 1.0

    common = {
        "w9qk": w9qk.astype(bf), "w9v": w9v.astype(bf), "wyd": wyd.astype(bf),
        "wv1x1": np.ascontiguousarray(qkv_w[256:].T).astype(bf),
        "saw1t": saw1tp.astype(bf),
        "w2rep": w2rep.astype(bf), "w3rep": w3rep.astype(bf),
        "spw1t": np.asarray(sp_w1, f32).T.copy(),
        "spw2t": np.asarray(sp_w2, f32).T.copy(),
        "spw3t": np.asarray(sp_w3, f32).T.copy(),
        "projt": np.asarray(proj_w, f32).T.copy(),
        "dwvw": np.ascontiguousarray(dw[256:]),
        "dwyw": np.ascontiguousarray(dwy),
        "consts": consts,
    }
    in_maps = []
    for i in range(NCORES):
        m = dict(common)
        m["xh"] = np.ascontiguousarray(xp[:, :, 32 * i:32 * i + HH, :])
        m["yh"] = np.ascontiguousarray(yp[:, :, 32 * i:32 * i + HH, :])
        in_maps.append(m)
    return in_maps


def kernel(**inputs):
    if "nc" not in _CACHE:
        _CACHE["nc"] = build_nc()
    nc = _CACHE["nc"]
    in_maps = _prep_host(**inputs)
    res = run_bass_kernel_spmd(nc, in_maps, core_ids=list(range(NCORES)))
    shards = [res.results[i]["out"] for i in range(NCORES)]
    return np.concatenate(shards, axis=2).astype(np.float32)


# revision 10
# speedup vs baseline: 2.7715x; 1.0670x over previous
"""Cross-Spatial-Attention Trainium2 kernel (8 NeuronCores, spatial sharding).

v2: engine-balanced. TensorE keeps the QK 9-tap fused conv (transposed
layout) + gram + SA gate + projections; the depthwise work for v and
dwconv(y) moves to the Vector/GpSimd engines as per-channel
multiply-accumulate passes (tensor_scalar 4x + tensor_tensor 2x, fp16),
fed by a cheap 1x1 conv for v_pre. The v-mean needed by the stats
AllReduce is computed from window sums of v_pre (row-sum side path) so
each batch's AllReduce fires right after its QK gram; batch1's
y-depthwise stays on TensorE as filler inside the AllReduce window.
"""

import numpy as np
from contextlib import ExitStack

import concourse.bass as bass
import concourse.bacc as bacc
import concourse.tile as tile
from concourse import mybir
from concourse.bass_utils import run_bass_kernel_spmd

FP32 = mybir.dt.float32
FP16 = mybir.dt.float16
BF16 = mybir.dt.bfloat16
AF = mybir.ActivationFunctionType
ALU = mybir.AluOpType

B, C, H, W = 2, 128, 256, 256
HD, DH = 8, 16
NCORES = 8
RPC = H // NCORES            # 32 rows per core
HH, WW = RPC + 2, W + 2      # 34 x 258 halo'd band
FREE = HH * WW               # 8772
NLOC = RPC * W               # 8192 output positions per band per batch
NCH_T = NLOC // 128          # 64 transposed chunks
NCH_A = NLOC // 512          # 16 layout-A chunks
NTOT = float(H * W)          # global spatial size

# tap engine assignment: per (tensor, batch) a list of 9 entries
# 'd' = DVE ts+tt, 'g' = DVE ts + GpSimd tt, 't' = TensorE dense fold
V_TAPS = {0: list("ddddddddd"), 1: list("ddddddddd")}
Y_TAPS = {0: list("ddddddddd"), 1: list("ddddddddd")}

TMPH = NLOC // 2             # DVE tap chunk


def _emit(tc, io):
    nc = tc.nc
    ctx = ExitStack()

    wpool = ctx.enter_context(tc.tile_pool(name="wpool", bufs=1))
    xpool = ctx.enter_context(tc.tile_pool(name="xpool", bufs=2))
    ypool = ctx.enter_context(tc.tile_pool(name="ypool", bufs=2))
    vppool = ctx.enter_context(tc.tile_pool(name="vppool", bufs=2))
    vpool = ctx.enter_context(tc.tile_pool(name="vpool", bufs=2))
    y2pool = ctx.enter_context(tc.tile_pool(name="y2pool", bufs=2))
    tmppool = ctx.enter_context(tc.tile_pool(name="tmppool", bufs=1))
    spool = ctx.enter_context(tc.tile_pool(name="spool", bufs=1))
    rpool = ctx.enter_context(tc.tile_pool(name="rpool", bufs=4))
    mpool = ctx.enter_context(tc.tile_pool(name="mpool", bufs=1))
    opool = ctx.enter_context(tc.tile_pool(name="opool", bufs=2))
    psA = ctx.enter_context(tc.tile_pool(name="psA", bufs=2, space="PSUM"))
    psQK = ctx.enter_context(tc.tile_pool(name="psQK", bufs=4, space="PSUM"))
    psG = ctx.enter_context(tc.tile_pool(name="psG", bufs=1, space="PSUM"))
    dpool = ctx.enter_context(tc.tile_pool(name="dram", bufs=4, space="DRAM"))

    def dma(dst, src):
        nc.sync.dma_start(out=dst, in_=src)

    def wload(name, shape, dt=BF16):
        t = wpool.tile(shape, dt, tag=name)
        dma(t[:], io[name][:])
        return t

    saw1t = wload("saw1t", [128, 32])
    w2rep = wload("w2rep", [128, 32])
    w3rep = wload("w3rep", [128, 1])
    xt0 = xpool.tile([128, FREE], BF16, tag="x")
    yt0 = ypool.tile([128, FREE], BF16, tag="y")
    dma(yt0[:], io["yh"][0].rearrange("c h w -> c (h w)"))
    dma(xt0[:], io["xh"][0].rearrange("c h w -> c (h w)"))
    w9qk = wload("w9qk", [128, 9 * 256])     # tap t at cols [256t:256t+256]
    wv1x1 = wload("wv1x1", [128, 128])       # v 1x1: [ic, oc]
    dwvw = wload("dwvw", [128, 9], FP32)     # v depthwise tap weights
    dwyw = wload("dwyw", [128, 9], FP32)     # y depthwise tap weights
    consts = wload("consts", [128, 386], FP32)
    need_w9v = any(s == "t" for b in range(B) for s in V_TAPS[b])
    w9v = wload("w9v", [128, 9 * 128]) if need_w9v else None
    need_wyd = any(s == "t" for b in range(B) for s in Y_TAPS[b])
    wyd = wload("wyd", [128, 9 * 128]) if need_wyd else None
    spw1t = wload("spw1t", [128, 16], FP32)
    spw2t = wload("spw2t", [16, 16], FP32)
    spw3t = wload("spw3t", [16, 128], FP32)
    projt = wload("projt", [128, 128], FP32)
    eye = consts[:, 0:128]
    bdmask = consts[:, 128:256]
    tempp = consts[:, 256:257]
    onesrow = consts[0:1, 257:385]

    xts, yts, vts, y2ts, saTs = [], [], [], [], []
    arreses = []

    # ---------------- helpers ----------------
    def sa_gate(b, yt):
        """spatial-attention gate -> saT [128, 64] (col j = chunk j)"""
        s1 = spool.tile([128, 2048], BF16, tag="s1")
        s2 = s1
        for g in range(4):
            ps1 = psA.tile([128, 512], FP32, tag="a")
            for k in range(4):
                nn = 4 * g + k
                r0 = 2 * nn
                yv = yt[:].rearrange("p (h w) -> p h w", h=HH)[
                    :, r0 + 1:r0 + 3, 1:257]
                nc.tensor.matmul(ps1[32 * k:32 * k + 32, :], saw1t[:, :], yv,
                                 start=True, stop=True,
                                 tile_position=(0, 32 * k)).annotate("mm_sa")
            if g % 2 == 0:
                nc.vector.tensor_scalar_max(s1[:, 512 * g:512 * g + 512], ps1[:, :], 0.0)
            else:
                nc.scalar.activation(s1[:, 512 * g:512 * g + 512], ps1[:, :], AF.Relu)
        for g in range(4):
            ps2 = psA.tile([128, 512], FP32, tag="a")
            for k in range(4):
                nc.tensor.matmul(ps2[32 * k:32 * k + 32, :],
                                 w2rep[32 * k:32 * k + 16, :],
                                 s1[32 * k:32 * k + 16, 512 * g:512 * g + 512],
                                 start=True, stop=True,
                                 tile_position=(32 * k, 32 * k))
            if g % 2 == 0:
                nc.vector.tensor_scalar_max(s2[:, 512 * g:512 * g + 512], ps2[:, :], 0.0)
            else:
                nc.scalar.activation(s2[:, 512 * g:512 * g + 512], ps2[:, :], AF.Relu)
        saT_ps = psQK.tile([128, 64], FP32, tag="qk")
        for j in range(NCH_T):
            nn, off = j // 4, (j % 4) * 128
            g, k = nn // 4, nn % 4
            nc.tensor.matmul(saT_ps[:, j:j + 1],
                             s2[32 * k:32 * k + 16,
                                512 * g + off:512 * g + off + 128],
                             w3rep[32 * k:32 * k + 16, :],
                             start=True, stop=True, tile_position=(32 * k, 0))
        saT = mpool.tile([128, 64], FP32, tag="saT")
        nc.scalar.activation(saT[:], saT_ps[:], AF.Sigmoid)
        return saT

    def v1x1(b, xt):
        """v_pre = Wv @ x over the halo'd band -> [128, FREE] fp16"""
        vp = vppool.tile([128, FREE], FP16, tag="vp")
        c0 = 0
        while c0 < FREE:
            w = min(512, FREE - c0)
            pv = psA.tile([128, 512], FP32, tag="a")
            nc.tensor.matmul(pv[:, 0:w], wv1x1[:, :], xt[:, c0:c0 + w],
                             start=True, stop=True).annotate("mm_v1x1")
            nc.scalar.copy(vp[:, c0:c0 + w], pv[:, 0:w]).annotate("cp_vp")
            c0 += w
        return vp

    def vsum_side(b, vp):
        """vsum[c] = sum over band of v (exact, via window sums of v_pre)."""
        vv = vp[:].rearrange("p (h w) -> p h w", h=HH)

        def edge(k):
            return vv[:, :, k:k + 1].rearrange("p h w -> p (h w)")

        fr = mpool.tile([128, 34], FP16, tag="fr")
        nc.vector.tensor_reduce(fr[:], vv[:, :, :], mybir.AxisListType.X,
                                ALU.add).annotate("vsum_red")
        # rs block tj at cols [34*tj : 34*tj+34]: row sums over cols tj..tj+255
        rs = mpool.tile([128, 102], FP32, tag="rs")
        pairs = [(256, 257), (0, 257), (0, 1)]
        for tj, (ka, kb) in enumerate(pairs):
            nc.vector.tensor_tensor(rs[:, 34 * tj:34 * tj + 34], fr[:],
                                    edge(ka), ALU.subtract)
            nc.vector.tensor_tensor(rs[:, 34 * tj:34 * tj + 34],
                                    rs[:, 34 * tj:34 * tj + 34],
                                    edge(kb), ALU.subtract)
        rs3 = rs[:].rearrange("p (t r) -> p t r", t=3)
        tj_tot = mpool.tile([128, 3], FP32, tag="tjt")
        nc.vector.tensor_reduce(tj_tot[:], rs3, mybir.AxisListType.X, ALU.add)
        # ws[3*ti+tj] = tj_tot[tj] - rs[tj, ex1(ti)] - rs[tj, ex2(ti)]
        ex = [(32, 33), (0, 33), (0, 1)]
        ws = mpool.tile([128, 9], FP32, tag="ws")
        for ti in range(3):
            a_, b_ = ex[ti]
            ra = rs3[:, :, a_:a_ + 1].rearrange("p t r -> p (t r)")
            rb = rs3[:, :, b_:b_ + 1].rearrange("p t r -> p (t r)")
            nc.vector.tensor_tensor(ws[:, 3 * ti:3 * ti + 3], tj_tot[:],
                                    ra, ALU.subtract)
            nc.vector.tensor_tensor(ws[:, 3 * ti:3 * ti + 3],
                                    ws[:, 3 * ti:3 * ti + 3],
                                    rb, ALU.subtract)
        wsw = mpool.tile([128, 9], FP32, tag="wsw")
        nc.vector.tensor_tensor(wsw[:], ws[:], dwvw[:], ALU.mult)
        vsum = mpool.tile([128, 1], FP32, tag="vsum")
        nc.vector.tensor_reduce(vsum[:], wsw[:], mybir.AxisListType.X, ALU.add)
        return vsum

    def qk_gram(b, xt, saT):
        """QK 9-tap fused conv in transposed layout + gram accumulation."""
        G = psG.tile([128, 256], FP32, tag="G")
        G2 = psG.tile([128, 128], FP32, tag="G2")
        for j in range(NCH_T):
            r, c0 = j // 2, (j % 2) * 128
            pqk = psQK.tile([128, 256], FP32, tag="qk")
            for t in range(9):
                ti, tj = t // 3, t % 3
                base = (r + ti) * WW + c0 + tj
                nc.tensor.matmul(pqk[:, :], xt[:, base:base + 128],
                                 w9qk[:, 256 * t:256 * t + 256],
                                 start=(t == 0), stop=(t == 8)).annotate("mm_qk")
            rt = rpool.tile([128, 256], BF16, tag="ring")
            nc.vector.tensor_scalar_mul(rt[:, 0:128], pqk[:, 0:128],
                                        saT[:, j:j + 1]).annotate("cp_rtq")
            nc.scalar.copy(rt[:, 128:256], pqk[:, 128:256]).annotate("cp_rtk")
            nc.tensor.matmul(G[:, 0:256], rt[:, 0:128], rt[:, 0:256],
                             start=(j == 0), stop=(j == NCH_T - 1),
                             skip_group_check=True).annotate("mm_gram")
            nc.tensor.matmul(G2[:, :], rt[:, 128:256], rt[:, 128:256],
                             start=(j == 0), stop=(j == NCH_T - 1),
                             skip_group_check=True).annotate("mm_gram")
        return G, G2

    def stage_stats(b, G, G2, vsum):
        """arst [128, 131]: [Gqk | qd | kd | vsum]"""
        arst = mpool.tile([128, 131], FP32, tag=f"arst{b}")
        junk = mpool.tile([128, 128], FP32, tag="junk")
        nc.vector.tensor_copy(arst[:, 0:128], G[:, 128:256])
        nc.vector.scalar_tensor_tensor(junk[:], G[:, 0:128], 1.0, eye,
                                       ALU.mult, ALU.mult,
                                       accum_out=arst[:, 128:129])
        nc.vector.scalar_tensor_tensor(junk[:], G2[:, :], 1.0, eye,
                                       ALU.mult, ALU.mult,
                                       accum_out=arst[:, 129:130])
        nc.vector.tensor_copy(arst[:, 130:131], vsum[:])
        return arst

    def issue_ar(b, arst):
        din = dpool.tile([128, 131], FP32, tag=f"din{b}")
        dout = dpool.tile([128, 131], FP32, tag=f"dout{b}")
        dma(din[:], arst[:])
        nc.gpsimd.collective_compute(
            "AllReduce", ALU.add,
            replica_groups=[list(range(NCORES))],
            ins=[din[:].opt()], outs=[dout[:].opt()])
        arres = mpool.tile([128, 131], FP32, tag=f"arres{b}")
        dma(arres[:], dout[:])
        return arres

    def taps(spec, src, dwv, acc_pool, acc_tag):
        """depthwise 3x3 over halo'd src [128, FREE] -> acc [128, NLOC] fp16."""
        sv = src[:].rearrange("p (h w) -> p h w", h=HH)
        acc = acc_pool.tile([128, NLOC], FP16, tag=acc_tag)
        d_taps = [t for t in range(9) if spec[t] == "d"]
        g_taps = [t for t in range(9) if spec[t] == "G"]
        te_taps = [t for t in range(9) if spec[t] == "t"]

        def shifted(t, c0, w):
            # column window [c0, c0+w) of the band output, rows 0..31
            ti, tj = t // 3, t % 3
            r0, cw = c0 // W, c0 % W
            nr = w // W
            return sv[:, ti + r0:ti + r0 + nr, tj + cw:tj + cw + W]

        first = True
        for t in d_taps:
            for h in range(NLOC // TMPH):
                c0 = h * TMPH
                dst = acc[:, c0:c0 + TMPH]
                dstv = dst.rearrange("p (h w) -> p h w", h=TMPH // W)
                if first:
                    nc.vector.tensor_scalar_mul(dstv, shifted(t, c0, TMPH),
                                                dwv[:, t:t + 1]).annotate("tap_ts")
                else:
                    tmp = tmppool.tile([128, TMPH], FP16, tag="tmp")
                    tmpv = tmp[:].rearrange("p (h w) -> p h w", h=TMPH // W)
                    nc.vector.tensor_scalar_mul(tmpv, shifted(t, c0, TMPH),
                                                dwv[:, t:t + 1]).annotate("tap_ts")
                    nc.vector.tensor_tensor(dst, dst, tmp[:],
                                            ALU.add).annotate("tap_tt")
            first = False
        return acc, te_taps

    def taps_te(b, src, w9, te_taps, acc, merge):
        """TE dense-fold taps over halo'd src, baseline-style; merge into acc."""
        if not te_taps:
            return
        sv = src[:].rearrange("p (h w) -> p h w", h=HH)
        for nn in range(NCH_A):
            r0 = 2 * nn
            py = psA.tile([128, 512], FP32, tag="a")
            for i, t in enumerate(te_taps):
                ti, tj = t // 3, t % 3
                xv = sv[:, r0 + ti:r0 + ti + 2, tj:tj + 256]
                nc.tensor.matmul(py[:, :], w9[:, 128 * t:128 * t + 128], xv,
                                 start=(i == 0),
                                 stop=(i == len(te_taps) - 1)).annotate("mm_yte")
            if merge:
                nc.vector.tensor_tensor(acc[:, 512 * nn:512 * nn + 512],
                                        acc[:, 512 * nn:512 * nn + 512],
                                        py[:, :], ALU.add).annotate("cp_te")
            else:
                nc.scalar.copy(acc[:, 512 * nn:512 * nn + 512],
                               py[:, :]).annotate("cp_te")

    def post_ar(b, arres):
        """norms -> softmax -> Meff/p2t; returns (mefft fp16, p2t fp16)"""
        rqk = mpool.tile([128, 2], FP32, tag="rqk")
        srt = mpool.tile([128, 2], FP32, tag="srt")
        dcat = arres[:, 128:130]
        nc.scalar.activation(srt[:], dcat, AF.Sqrt)
        nc.vector.tensor_scalar_max(srt[:], srt[:], 1e-12)
        nc.vector.reciprocal(rqk[:], srt[:])
        r2 = mpool.tile([128, 2], FP32, tag="r2")
        nc.vector.tensor_tensor(r2[:], rqk[:], rqk[:], ALU.mult)
        nc.vector.tensor_tensor(r2[:], r2[:], dcat, ALU.mult)
        nc.vector.tensor_scalar(r2[:], r2[:], -0.5, 1.5, ALU.mult, ALU.add)
        nc.vector.tensor_tensor(rqk[:], rqk[:], r2[:], ALU.mult)
        rqt = mpool.tile([128, 1], FP32, tag="rqt")
        nc.vector.tensor_tensor(rqt[:], rqk[:, 0:1], tempp, ALU.mult)

        ps1 = psA.tile([128, 128], FP32, tag="a")
        nc.tensor.matmul(ps1[0:1, :], rqk[:, 1:2], eye, start=True, stop=True)
        rkrow = mpool.tile([1, 128], FP32, tag="rkrow")
        nc.scalar.copy(rkrow[:], ps1[0:1, :])
        ps2 = psA.tile([128, 128], FP32, tag="a")
        nc.tensor.matmul(ps2[:, :], onesrow, rkrow[:], start=True, stop=True)

        gh = mpool.tile([128, 128], FP32, tag="gh")
        nc.vector.scalar_tensor_tensor(gh[:], arres[:, 0:128], rqt[:, 0:1],
                                       ps2[:, :], ALU.mult, ALU.mult)
        sm = mpool.tile([128, 128], FP32, tag="sm")
        nc.scalar.activation(sm[:], gh[:], AF.Exp)
        rs_ = mpool.tile([128, 1], FP32, tag="rssm")
        nc.vector.scalar_tensor_tensor(sm[:], sm[:], 1.0, bdmask,
                                       ALU.mult, ALU.mult, accum_out=rs_[:])
        nc.vector.reciprocal(rs_[:], rs_[:])
        attn = mpool.tile([128, 128], FP32, tag="attn")
        nc.vector.tensor_scalar_mul(attn[:], sm[:], rs_[:, 0:1])

        psM = psA.tile([128, 128], FP32, tag="a")
        nc.tensor.matmul(psM[:, :], attn[:], projt[:], start=True, stop=True)
        mefft = mpool.tile([128, 128], FP16, tag="mefft")
        nc.scalar.copy(mefft[:], psM[:, :])

        psT = psA.tile([128, 128], FP32, tag="a")
        nc.tensor.transpose(psT[:, :], attn[:], eye)
        attnt = mpool.tile([128, 128], FP32, tag="attnt")
        nc.vector.tensor_copy(attnt[:], psT[:, :])
        psP = psA.tile([128, 1], FP32, tag="a")
        nc.tensor.matmul(psP[:, :], attnt[:], arres[:, 130:131],
                         start=True, stop=True)
        pooled = mpool.tile([128, 1], FP32, tag="pooled")
        nc.scalar.activation(pooled[:], psP[:, :], AF.Copy, scale=1.0 / NTOT)

        psg1 = psA.tile([16, 1], FP32, tag="a")
        nc.tensor.matmul(psg1[:, :], spw1t[:], pooled[:], start=True, stop=True)
        g1 = mpool.tile([16, 1], FP32, tag="g1")
        nc.scalar.activation(g1[:], psg1[:, :], AF.Gelu)
        psg2 = psA.tile([16, 1], FP32, tag="a")
        nc.tensor.matmul(psg2[:, :], spw2t[:], g1[:], start=True, stop=True)
        g2 = mpool.tile([16, 1], FP32, tag="g2")
        nc.scalar.activation(g2[:], psg2[:, :], AF.Gelu)
        psg3 = psA.tile([128, 1], FP32, tag="a")
        nc.tensor.matmul(psg3[:, :], spw3t[:], g2[:], start=True, stop=True)
        spec = mpool.tile([128, 1], FP32, tag="spec")
        nc.scalar.activation(spec[:], psg3[:, :], AF.Sigmoid)

        p2t = mpool.tile([128, 128], FP16, tag="p2t")
        nc.vector.tensor_scalar_mul(p2t[:], projt[:], spec[:, 0:1])
        return mefft, p2t

    def final_proj(b, mefft, p2t, vt, y2t):
        out2d = io["out"][b].rearrange("c h w -> c (h w)")
        for nn in range(NCH_A):
            pf = psA.tile([128, 512], FP32, tag="a")
            nc.tensor.matmul(pf[:, :], mefft[:],
                             vt[:, 512 * nn:512 * nn + 512],
                             start=True, stop=False).annotate("mm_proj")
            nc.tensor.matmul(pf[:, :], p2t[:],
                             y2t[:, 512 * nn:512 * nn + 512],
                             start=False, stop=True).annotate("mm_proj")
            ot = opool.tile([128, 512], FP16, tag="ot")
            nc.scalar.copy(ot[:], pf[:, :]).annotate("cp_ot")
            dma(out2d[:, 512 * nn:512 * nn + 512], ot[:])

    # ================= schedule =================
    with nc.allow_low_precision(reason="fp16 depthwise accumulation"):
        for b in range(B):
            if b == 0:
                xt, yt = xt0, yt0
            else:
                xt = xpool.tile([128, FREE], BF16, tag="x")
                yt = ypool.tile([128, FREE], BF16, tag="y")
                dma(yt[:], io["yh"][b].rearrange("c h w -> c (h w)"))
                dma(xt[:], io["xh"][b].rearrange("c h w -> c (h w)"))
            xts.append(xt)
            yts.append(yt)

            saT = sa_gate(b, yt)
            saTs.append(saT)
            vp = v1x1(b, xt)
            vsum = vsum_side(b, vp)
            G, G2 = qk_gram(b, xt, saT)
            arst = stage_stats(b, G, G2, vsum)
            arres = issue_ar(b, arst)
            arreses.append(arres)
            if b == 1:
                mp0 = post_ar(0, arreses[0])

            # DVE taps (no dep on the AllReduce; fill its window)
            vt, v_te = taps(V_TAPS[b], vp, dwvw, vpool, "vt")
            taps_te(b, xt, w9v, v_te, vt, merge=True)
            vts.append(vt)
            y_spec = Y_TAPS[b]
            if all(s == "t" for s in y_spec):
                y2t = y2pool.tile([128, NLOC], FP16, tag="y2t")
                taps_te(b, yt, wyd, list(range(9)), y2t, merge=False)
            else:
                y2t, y_te = taps(y_spec, yt, dwyw, y2pool, "y2t")
                taps_te(b, yt, wyd, y_te, y2t, merge=True)
            y2ts.append(y2t)

        final_proj(0, mp0[0], mp0[1], vts[0], y2ts[0])
        mefft1, p2t1 = post_ar(1, arreses[1])
        final_proj(1, mefft1, p2t1, vts[1], y2ts[1])

    ctx.close()


def build_nc():
    nc = bacc.Bacc("TRN2", target_bir_lowering=False, debug=False,
                   num_devices=NCORES)
    io = {}

    def inp(name, shape, dt):
        io[name] = nc.dram_tensor(name, shape, dt, kind="ExternalInput")

    inp("xh", [B, C, HH, WW], BF16)
    inp("yh", [B, C, HH, WW], BF16)
    inp("w9qk", [128, 9 * 256], BF16)
    inp("w9v", [128, 9 * 128], BF16)
    inp("wyd", [128, 9 * 128], BF16)
    inp("wv1x1", [128, 128], BF16)
    inp("saw1t", [128, 32], BF16)
    inp("w2rep", [128, 32], BF16)
    inp("w3rep", [128, 1], BF16)
    inp("spw1t", [128, 16], FP32)
    inp("spw2t", [16, 16], FP32)
    inp("spw3t", [16, 128], FP32)
    inp("projt", [128, 128], FP32)
    inp("dwvw", [128, 9], FP32)
    inp("dwyw", [128, 9], FP32)
    inp("consts", [128, 386], FP32)
    io["out"] = nc.dram_tensor("out", [B, C, RPC, W], FP16, kind="ExternalOutput")

    with tile.TileContext(nc) as tc:
        _emit(tc, io)
    nc.finalize()
    return nc


_CACHE = {}


def _prep_host(x, y, qkv_w, qkv_dw_w, proj_w, sa_w1, sa_w2, sa_w3,
               sp_w1, sp_w2, sp_w3, dw_w, temperature):
    import ml_dtypes
    bf = ml_dtypes.bfloat16
    f32 = np.float32

    x = np.asarray(x, f32)
    y = np.asarray(y, f32)
    xp = np.zeros((B, C, H + 2, W + 2), f32)
    xp[:, :, 1:H + 1, 1:W + 1] = x
    yp = np.zeros((B, C, H + 2, W + 2), f32)
    yp[:, :, 1:H + 1, 1:W + 1] = y
    xp = xp.astype(bf)
    yp = yp.astype(bf)

    qkv_w = np.asarray(qkv_w, f32)
    dw = np.asarray(qkv_dw_w, f32).reshape(3 * C, 9)
    w9qk = np.concatenate(
        [(qkv_w[:256] * dw[:256, t:t + 1]).T for t in range(9)], axis=1)
    w9v = np.concatenate(
        [(qkv_w[256:] * dw[256:, t:t + 1]).T for t in range(9)], axis=1)
    dwy = np.asarray(dw_w, f32).reshape(C, 9)
    wyd = np.concatenate(
        [np.diag(dwy[:, t]) for t in range(9)], axis=1)

    w2rep = np.zeros((128, 32), f32)
    w3rep = np.zeros((128, 1), f32)
    for k in range(4):
        w2rep[32 * k:32 * k + 16, 0:16] = np.asarray(sa_w2, f32).T
        w3rep[32 * k:32 * k + 16] = np.asarray(sa_w3, f32).T
    saw1tp = np.zeros((128, 32), f32)
    saw1tp[:, 0:16] = np.asarray(sa_w1, f32).T

    consts = np.zeros((128, 386), f32)
    consts[:, 0:128] = np.eye(128, dtype=f32)
    ci = np.arange(128) // DH
    consts[:, 128:256] = (ci[:, None] == ci[None, :]).astype(f32)
    consts[:, 256] = np.asarray(temperature, f32).reshape(HD)[ci]
    consts[0, 257:385] = 1.0

    common = {
        "w9qk": w9qk.astype(bf), "w9v": w9v.astype(bf), "wyd": wyd.astype(bf),
        "wv1x1": np.ascontiguousarray(qkv_w[256:].T).astype(bf),
        "saw1t": saw1tp.astype(bf),
        "w2rep": w2rep.astype(bf), "w3rep": w3rep.astype(bf),
        "spw1t": np.asarray(sp_w1, f32).T.copy(),
        "spw2t": np.asarray(sp_w2, f32).T.copy(),
        "spw3t": np.asarray(sp_w3, f32).T.copy(),
        "projt": np.asarray(proj_w, f32).T.copy(),
        "dwvw": np.ascontiguousarray(dw[256:]),
        "dwyw": np.ascontiguousarray(dwy),
        "consts": consts,
    }
    in_maps = []
    for i in range(NCORES):
        m = dict(common)
        m["xh"] = np.ascontiguousarray(xp[:, :, 32 * i:32 * i + HH, :])
        m["yh"] = np.ascontiguousarray(yp[:, :, 32 * i:32 * i + HH, :])
        in_maps.append(m)
    return in_maps


def kernel(**inputs):
    if "nc" not in _CACHE:
        _CACHE["nc"] = build_nc()
    nc = _CACHE["nc"]
    in_maps = _prep_host(**inputs)
    res = run_bass_kernel_spmd(nc, in_maps, core_ids=list(range(NCORES)))
    shards = [res.results[i]["out"] for i in range(NCORES)]
    return np.concatenate(shards, axis=2).astype(np.float32)


# revision 16
# speedup vs baseline: 2.8832x; 1.0403x over previous
"""Cross-Spatial-Attention Trainium2 kernel (8 NeuronCores, spatial sharding).

v2: engine-balanced. TensorE keeps the QK 9-tap fused conv (transposed
layout) + gram + SA gate + projections; the depthwise work for v and
dwconv(y) moves to the Vector/GpSimd engines as per-channel
multiply-accumulate passes (tensor_scalar 4x + tensor_tensor 2x, fp16),
fed by a cheap 1x1 conv for v_pre. The v-mean needed by the stats
AllReduce is computed from window sums of v_pre (row-sum side path) so
each batch's AllReduce fires right after its QK gram; batch1's
y-depthwise stays on TensorE as filler inside the AllReduce window.
"""

import numpy as np
from contextlib import ExitStack

import concourse.bass as bass
import concourse.bacc as bacc
import concourse.tile as tile
from concourse import mybir
from concourse.bass_utils import run_bass_kernel_spmd

FP32 = mybir.dt.float32
FP16 = mybir.dt.float16
BF16 = mybir.dt.bfloat16
AF = mybir.ActivationFunctionType
ALU = mybir.AluOpType

B, C, H, W = 2, 128, 256, 256
HD, DH = 8, 16
NCORES = 8
RPC = H // NCORES            # 32 rows per core
HH, WW = RPC + 2, W + 2      # 34 x 258 halo'd band
FREE = HH * WW               # 8772
NLOC = RPC * W               # 8192 output positions per band per batch
NCH_T = NLOC // 128          # 64 transposed chunks
NCH_A = NLOC // 512          # 16 layout-A chunks
NTOT = float(H * W)          # global spatial size

# tap engine assignment: per (tensor, batch) a list of 9 entries
# 'd' = DVE ts+tt, 'g' = DVE ts + GpSimd tt, 't' = TensorE dense fold
V_TAPS = {0: list("dddddggff"), 1: list("dddddddff")}
Y_TAPS = {0: list("dddddggff"), 1: list("dddddddff")}
FILL_EVERY = 3               # emit one DVE filler per this many QK chunks

TMPH = NLOC // 2             # DVE tap chunk
TMPG = NLOC // 4             # gpsimd tap chunk


def _emit(tc, io):
    nc = tc.nc
    ctx = ExitStack()

    wpool = ctx.enter_context(tc.tile_pool(name="wpool", bufs=1))
    xpool = ctx.enter_context(tc.tile_pool(name="xpool", bufs=2))
    ypool = ctx.enter_context(tc.tile_pool(name="ypool", bufs=2))
    vppool = ctx.enter_context(tc.tile_pool(name="vppool", bufs=2))
    vpool = ctx.enter_context(tc.tile_pool(name="vpool", bufs=2))
    y2pool = ctx.enter_context(tc.tile_pool(name="y2pool", bufs=2))
    tmppool = ctx.enter_context(tc.tile_pool(name="tmppool", bufs=1))
    spool = ctx.enter_context(tc.tile_pool(name="spool", bufs=1))
    rpool = ctx.enter_context(tc.tile_pool(name="rpool", bufs=4))
    mpool = ctx.enter_context(tc.tile_pool(name="mpool", bufs=1))
    opool = ctx.enter_context(tc.tile_pool(name="opool", bufs=2))
    psA = ctx.enter_context(tc.tile_pool(name="psA", bufs=2, space="PSUM"))
    psQK = ctx.enter_context(tc.tile_pool(name="psQK", bufs=4, space="PSUM"))
    psG = ctx.enter_context(tc.tile_pool(name="psG", bufs=1, space="PSUM"))
    dpool = ctx.enter_context(tc.tile_pool(name="dram", bufs=4, space="DRAM"))

    def dma(dst, src):
        nc.sync.dma_start(out=dst, in_=src)

    def wload(name, shape, dt=BF16):
        t = wpool.tile(shape, dt, tag=name)
        dma(t[:], io[name][:])
        return t

    saw1t = wload("saw1t", [128, 32])
    w2rep = wload("w2rep", [128, 32])
    w3rep = wload("w3rep", [128, 1])
    xt0 = xpool.tile([128, FREE], BF16, tag="x")
    yt0 = ypool.tile([128, FREE], BF16, tag="y")
    dma(yt0[:], io["yh"][0].rearrange("c h w -> c (h w)"))
    dma(xt0[:], io["xh"][0].rearrange("c h w -> c (h w)"))
    w9qk = wload("w9qk", [128, 9 * 256])     # tap t at cols [256t:256t+256]
    wv1x1 = wload("wv1x1", [128, 128])       # v 1x1: [ic, oc]
    dwvw = wload("dwvw", [128, 9], FP32)     # v depthwise tap weights
    dwyw = wload("dwyw", [128, 9], FP32)     # y depthwise tap weights
    consts = wload("consts", [128, 386], FP32)
    need_w9v = any(s == "t" for b in range(B) for s in V_TAPS[b])
    w9v = wload("w9v", [128, 9 * 128]) if need_w9v else None
    need_w9vf = any(s == "f" for b in range(B) for s in V_TAPS[b])
    w9vf = wload("w9vf", [128, 9 * 128], FP16) if need_w9vf else None
    need_wyd = any(s == "t" for b in range(B) for s in Y_TAPS[b])
    wyd = wload("wyd", [128, 9 * 128]) if need_wyd else None
    spw1t = wload("spw1t", [128, 16], FP32)
    spw2t = wload("spw2t", [16, 16], FP32)
    spw3t = wload("spw3t", [16, 128], FP32)
    projt = wload("projt", [128, 128], FP32)
    eye = consts[:, 0:128]
    bdmask = consts[:, 128:256]
    tempp = consts[:, 256:257]
    onesrow = consts[0:1, 257:385]

    xts, yts, vts, y2ts, saTs = [], [], [], [], []
    vfs, yfs = [], []
    arreses = []

    # ---------------- helpers ----------------
    def sa_gate(b, yt):
        """spatial-attention gate -> saT [128, 64] (col j = chunk j)"""
        s1 = spool.tile([128, 2048], BF16, tag="s1")
        s2 = s1
        for g in range(4):
            ps1 = psA.tile([128, 512], FP32, tag="a")
            for k in range(4):
                nn = 4 * g + k
                r0 = 2 * nn
                yv = yt[:].rearrange("p (h w) -> p h w", h=HH)[
                    :, r0 + 1:r0 + 3, 1:257]
                nc.tensor.matmul(ps1[32 * k:32 * k + 32, :], saw1t[:, :], yv,
                                 start=True, stop=True,
                                 tile_position=(0, 32 * k)).annotate("mm_sa")
            if g % 2 == 0:
                nc.vector.tensor_scalar_max(s1[:, 512 * g:512 * g + 512], ps1[:, :], 0.0)
            else:
                nc.scalar.activation(s1[:, 512 * g:512 * g + 512], ps1[:, :], AF.Relu)
        for g in range(4):
            ps2 = psA.tile([128, 512], FP32, tag="a")
            for k in range(4):
                nc.tensor.matmul(ps2[32 * k:32 * k + 32, :],
                                 w2rep[32 * k:32 * k + 16, :],
                                 s1[32 * k:32 * k + 16, 512 * g:512 * g + 512],
                                 start=True, stop=True,
                                 tile_position=(32 * k, 32 * k))
            if g % 2 == 0:
                nc.vector.tensor_scalar_max(s2[:, 512 * g:512 * g + 512], ps2[:, :], 0.0)
            else:
                nc.scalar.activation(s2[:, 512 * g:512 * g + 512], ps2[:, :], AF.Relu)
        saT_ps = psQK.tile([128, 64], FP32, tag="qk")
        for j in range(NCH_T):
            nn, off = j // 4, (j % 4) * 128
            g, k = nn // 4, nn % 4
            nc.tensor.matmul(saT_ps[:, j:j + 1],
                             s2[32 * k:32 * k + 16,
                                512 * g + off:512 * g + off + 128],
                             w3rep[32 * k:32 * k + 16, :],
                             start=True, stop=True, tile_position=(32 * k, 0))
        saT = mpool.tile([128, 64], FP32, tag="saT")
        nc.scalar.activation(saT[:], saT_ps[:], AF.Sigmoid)
        return saT

    def v1x1(b, xt):
        """v_pre = Wv @ x over the halo'd band -> [128, FREE] fp16"""
        vp = vppool.tile([128, FREE], FP16, tag="vp")
        c0 = 0
        while c0 < FREE:
            w = min(512, FREE - c0)
            pv = psA.tile([128, 512], FP32, tag="a")
            nc.tensor.matmul(pv[:, 0:w], wv1x1[:, :], xt[:, c0:c0 + w],
                             start=True, stop=True).annotate("mm_v1x1")
            nc.scalar.copy(vp[:, c0:c0 + w], pv[:, 0:w]).annotate("cp_vp")
            c0 += w
        return vp

    def vsum_side(b, vp):
        """vsum[c] = sum over band of v (exact, via window sums of v_pre).

        Returns (vsum tile, filler closures). The closures do the actual
        work (ACT row sums + small DVE combines) and must be popped before
        stage_stats(b) runs."""
        vv = vp[:].rearrange("p (h w) -> p h w", h=HH)

        def edge(k):
            return vv[:, :, k:k + 1].rearrange("p h w -> p (h w)")

        fr = mpool.tile([128, 34], FP32, tag=f"fr{b}")
        junkr = mpool.tile([128, 258], FP16, tag="junkr")
        vsum = mpool.tile([128, 1], FP32, tag=f"vsum{b}")
        closures = []

        def rows(r0, r1):
            def f():
                for r in range(r0, r1):
                    rowap = vv[:, r:r + 1, :].rearrange("p h w -> p (h w)")
                    nc.scalar.activation(junkr[:], rowap, AF.Copy,
                                         accum_out=fr[:, r:r + 1]).annotate("vsum_red")
            return f

        for r0 in range(0, HH, 6):
            closures.append(rows(r0, min(r0 + 6, HH)))

        def mini():
            # rs block tj at cols [34*tj:34*tj+34]: row sums over tj..tj+255
            rs = mpool.tile([128, 102], FP32, tag="rs")
            pairs = [(256, 257), (0, 257), (0, 1)]
            for tj, (ka, kb) in enumerate(pairs):
                nc.vector.tensor_tensor(rs[:, 34 * tj:34 * tj + 34], fr[:],
                                        edge(ka), ALU.subtract)
                nc.vector.tensor_tensor(rs[:, 34 * tj:34 * tj + 34],
                                        rs[:, 34 * tj:34 * tj + 34],
                                        edge(kb), ALU.subtract)
            rs3 = rs[:].rearrange("p (t r) -> p t r", t=3)
            tj_tot = mpool.tile([128, 3], FP32, tag="tjt")
            nc.vector.tensor_reduce(tj_tot[:], rs3, mybir.AxisListType.X, ALU.add)
            ex = [(32, 33), (0, 33), (0, 1)]
            ws = mpool.tile([128, 9], FP32, tag="ws")
            for ti in range(3):
                a_, b_ = ex[ti]
                ra = rs3[:, :, a_:a_ + 1].rearrange("p t r -> p (t r)")
                rb = rs3[:, :, b_:b_ + 1].rearrange("p t r -> p (t r)")
                nc.vector.tensor_tensor(ws[:, 3 * ti:3 * ti + 3], tj_tot[:],
                                        ra, ALU.subtract)
                nc.vector.tensor_tensor(ws[:, 3 * ti:3 * ti + 3],
                                        ws[:, 3 * ti:3 * ti + 3],
                                        rb, ALU.subtract)
            wsw = mpool.tile([128, 9], FP32, tag="wsw")
            nc.vector.tensor_tensor(wsw[:], ws[:], dwvw[:], ALU.mult)
            nc.vector.tensor_reduce(vsum[:], wsw[:], mybir.AxisListType.X,
                                    ALU.add)

        closures.append(mini)
        return vsum, closures

    def qk_gram(b, xt, saT, fillers):
        """QK 9-tap fused conv in transposed layout + gram accumulation."""
        G = psG.tile([128, 256], FP32, tag="G")
        G2 = psG.tile([128, 128], FP32, tag="G2")
        for j in range(NCH_T):
            if j % FILL_EVERY == FILL_EVERY - 1 and fillers:
                fillers.popleft()()
            r, c0 = j // 2, (j % 2) * 128
            pqk = psQK.tile([128, 256], FP32, tag="qk")
            for t in range(9):
                ti, tj = t // 3, t % 3
                base = (r + ti) * WW + c0 + tj
                nc.tensor.matmul(pqk[:, :], xt[:, base:base + 128],
                                 w9qk[:, 256 * t:256 * t + 256],
                                 start=(t == 0), stop=(t == 8)).annotate("mm_qk")
            rt = rpool.tile([128, 256], BF16, tag="ring")
            nc.vector.tensor_scalar_mul(rt[:, 0:128], pqk[:, 0:128],
                                        saT[:, j:j + 1]).annotate("cp_rtq")
            nc.scalar.copy(rt[:, 128:256], pqk[:, 128:256]).annotate("cp_rtk")
            nc.tensor.matmul(G[:, 0:256], rt[:, 0:128], rt[:, 0:256],
                             start=(j == 0), stop=(j == NCH_T - 1),
                             skip_group_check=True).annotate("mm_gram")
            nc.tensor.matmul(G2[:, :], rt[:, 128:256], rt[:, 128:256],
                             start=(j == 0), stop=(j == NCH_T - 1),
                             skip_group_check=True).annotate("mm_gram")
        return G, G2

    def stage_stats(b, G, G2, vsum):
        """arst [128, 131]: [Gqk | qd | kd | vsum]"""
        arst = mpool.tile([128, 131], FP32, tag=f"arst{b}")
        junk = mpool.tile([128, 128], FP32, tag="junk")
        nc.vector.tensor_copy(arst[:, 0:128], G[:, 128:256])
        nc.vector.scalar_tensor_tensor(junk[:], G[:, 0:128], 1.0, eye,
                                       ALU.mult, ALU.mult,
                                       accum_out=arst[:, 128:129])
        nc.vector.scalar_tensor_tensor(junk[:], G2[:, :], 1.0, eye,
                                       ALU.mult, ALU.mult,
                                       accum_out=arst[:, 129:130])
        nc.vector.tensor_copy(arst[:, 130:131], vsum[:])
        return arst

    def issue_ar(b, arst):
        din = dpool.tile([128, 131], FP32, tag=f"din{b}")
        dout = dpool.tile([128, 131], FP32, tag=f"dout{b}")
        dma(din[:], arst[:])
        nc.gpsimd.collective_compute(
            "AllReduce", ALU.add,
            replica_groups=[list(range(NCORES))],
            ins=[din[:].opt()], outs=[dout[:].opt()])
        arres = mpool.tile([128, 131], FP32, tag=f"arres{b}")
        dma(arres[:], dout[:])
        return arres

    def tap_fillers(spec, src, dwv, acc_pool, acc_tag):
        """depthwise 3x3 over halo'd src -> acc [128, NLOC] fp16.

        Returns (acc, te_taps, fillers): each filler is a closure emitting
        one half-width tap op pair; pop them in order."""
        sv = src[:].rearrange("p (h w) -> p h w", h=HH)
        acc = acc_pool.tile([128, NLOC], FP16, tag=acc_tag)
        d_taps = [t for t in range(9) if spec[t] == "d"]
        g_taps = [t for t in range(9) if spec[t] == "g"]
        te_taps = [t for t in range(9) if spec[t] == "t"]
        f_taps = [t for t in range(9) if spec[t] == "f"]

        def shifted(t, c0, w):
            ti, tj = t // 3, t % 3
            r0, cw = c0 // W, c0 % W
            nr = w // W
            return sv[:, ti + r0:ti + r0 + nr, tj + cw:tj + cw + W]

        fillers = []
        for i, t in enumerate(d_taps):
            for h in range(NLOC // TMPH):
                def f(t=t, h=h, first=(i == 0)):
                    c0 = h * TMPH
                    dst = acc[:, c0:c0 + TMPH]
                    dstv = dst.rearrange("p (h w) -> p h w", h=TMPH // W)
                    if first:
                        nc.vector.tensor_scalar_mul(
                            dstv, shifted(t, c0, TMPH),
                            dwv[:, t:t + 1]).annotate("tap_ts")
                    else:
                        tmp = tmppool.tile([128, TMPH], FP16, tag="tmp")
                        tmpv = tmp[:].rearrange("p (h w) -> p h w", h=TMPH // W)
                        nc.vector.tensor_scalar_mul(
                            tmpv, shifted(t, c0, TMPH),
                            dwv[:, t:t + 1]).annotate("tap_ts")
                        nc.vector.tensor_tensor(dst, dst, tmp[:],
                                                ALU.add).annotate("tap_tt")
                fillers.append(f)
        for t in g_taps:
            for h in range(NLOC // TMPH):
                def f(t=t, h=h):
                    for q in range(TMPH // TMPG):
                        c0 = h * TMPH + q * TMPG
                        dst = acc[:, c0:c0 + TMPG]
                        tmpg = tmppool.tile([128, TMPG], FP16, tag="tmpg")
                        tmpgv = tmpg[:].rearrange("p (h w) -> p h w",
                                                  h=TMPG // W)
                        nc.vector.tensor_scalar_mul(
                            tmpgv, shifted(t, c0, TMPG),
                            dwv[:, t:t + 1]).annotate("tap_gts")
                        nc.gpsimd.tensor_tensor(dst, dst, tmpg[:],
                                                ALU.add).annotate("tap_gtt")
                fillers.append(f)
        return acc, te_taps, f_taps, fillers

    def taps_te(b, src, w9, te_taps, acc, merge):
        """TE dense-fold taps over halo'd src, baseline-style; merge into acc."""
        if not te_taps:
            return
        sv = src[:].rearrange("p (h w) -> p h w", h=HH)
        for nn in range(NCH_A):
            r0 = 2 * nn
            py = psA.tile([128, 512], FP32, tag="a")
            for i, t in enumerate(te_taps):
                ti, tj = t // 3, t % 3
                xv = sv[:, r0 + ti:r0 + ti + 2, tj:tj + 256]
                nc.tensor.matmul(py[:, :], w9[:, 128 * t:128 * t + 128], xv,
                                 start=(i == 0),
                                 stop=(i == len(te_taps) - 1)).annotate("mm_yte")
            if merge:
                nc.vector.tensor_tensor(acc[:, 512 * nn:512 * nn + 512],
                                        acc[:, 512 * nn:512 * nn + 512],
                                        py[:, :], ALU.add).annotate("cp_te")
            else:
                nc.scalar.copy(acc[:, 512 * nn:512 * nn + 512],
                               py[:, :]).annotate("cp_te")

    def post_ar(b, arres):
        """norms -> softmax -> Meff/p2t; returns (mefft fp16, p2t fp16)"""
        rqk = mpool.tile([128, 2], FP32, tag="rqk")
        srt = mpool.tile([128, 2], FP32, tag="srt")
        dcat = arres[:, 128:130]
        nc.scalar.activation(srt[:], dcat, AF.Sqrt)
        nc.vector.tensor_scalar_max(srt[:], srt[:], 1e-12)
        nc.vector.reciprocal(rqk[:], srt[:])
        r2 = mpool.tile([128, 2], FP32, tag="r2")
        nc.vector.tensor_tensor(r2[:], rqk[:], rqk[:], ALU.mult)
        nc.vector.tensor_tensor(r2[:], r2[:], dcat, ALU.mult)
        nc.vector.tensor_scalar(r2[:], r2[:], -0.5, 1.5, ALU.mult, ALU.add)
        nc.vector.tensor_tensor(rqk[:], rqk[:], r2[:], ALU.mult)
        rqt = mpool.tile([128, 1], FP32, tag="rqt")
        nc.vector.tensor_tensor(rqt[:], rqk[:, 0:1], tempp, ALU.mult)

        ps1 = psA.tile([128, 128], FP32, tag="a")
        nc.tensor.matmul(ps1[0:1, :], rqk[:, 1:2], eye, start=True, stop=True)
        rkrow = mpool.tile([1, 128], FP32, tag="rkrow")
        nc.scalar.copy(rkrow[:], ps1[0:1, :])
        ps2 = psA.tile([128, 128], FP32, tag="a")
        nc.tensor.matmul(ps2[:, :], onesrow, rkrow[:], start=True, stop=True)

        gh = mpool.tile([128, 128], FP32, tag="gh")
        nc.vector.scalar_tensor_tensor(gh[:], arres[:, 0:128], rqt[:, 0:1],
                                       ps2[:, :], ALU.mult, ALU.mult)
        sm = mpool.tile([128, 128], FP32, tag="sm")
        nc.scalar.activation(sm[:], gh[:], AF.Exp)
        rs_ = mpool.tile([128, 1], FP32, tag="rssm")
        nc.vector.scalar_tensor_tensor(sm[:], sm[:], 1.0, bdmask,
                                       ALU.mult, ALU.mult, accum_out=rs_[:])
        nc.vector.reciprocal(rs_[:], rs_[:])
        attn = mpool.tile([128, 128], FP32, tag="attn")
        nc.vector.tensor_scalar_mul(attn[:], sm[:], rs_[:, 0:1])

        psM = psA.tile([128, 128], FP32, tag="a")
        nc.tensor.matmul(psM[:, :], attn[:], projt[:], start=True, stop=True)
        mefft = mpool.tile([128, 128], FP16, tag="mefft")
        nc.scalar.copy(mefft[:], psM[:, :])

        psT = psA.tile([128, 128], FP32, tag="a")
        nc.tensor.transpose(psT[:, :], attn[:], eye)
        attnt = mpool.tile([128, 128], FP32, tag="attnt")
        nc.vector.tensor_copy(attnt[:], psT[:, :])
        psP = psA.tile([128, 1], FP32, tag="a")
        nc.tensor.matmul(psP[:, :], attnt[:], arres[:, 130:131],
                         start=True, stop=True)
        pooled = mpool.tile([128, 1], FP32, tag="pooled")
        nc.scalar.activation(pooled[:], psP[:, :], AF.Copy, scale=1.0 / NTOT)

        psg1 = psA.tile([16, 1], FP32, tag="a")
        nc.tensor.matmul(psg1[:, :], spw1t[:], pooled[:], start=True, stop=True)
        g1 = mpool.tile([16, 1], FP32, tag="g1")
        nc.scalar.activation(g1[:], psg1[:, :], AF.Gelu)
        psg2 = psA.tile([16, 1], FP32, tag="a")
        nc.tensor.matmul(psg2[:, :], spw2t[:], g1[:], start=True, stop=True)
        g2 = mpool.tile([16, 1], FP32, tag="g2")
        nc.scalar.activation(g2[:], psg2[:, :], AF.Gelu)
        psg3 = psA.tile([128, 1], FP32, tag="a")
        nc.tensor.matmul(psg3[:, :], spw3t[:], g2[:], start=True, stop=True)
        spec = mpool.tile([128, 1], FP32, tag="spec")
        nc.scalar.activation(spec[:], psg3[:, :], AF.Sigmoid)

        p2t = mpool.tile([128, 128], FP16, tag="p2t")
        nc.vector.tensor_scalar_mul(p2t[:], projt[:], spec[:, 0:1])
        return mefft, p2t

    def fold_weights(b, mefft, p2t, vf_taps, yf_taps, xt, yt):
        """lhsT weights for proj-folded taps: K_t = w9vf_t^T @ mefft (v),
        M_t = p2t * dwy_t (y). Returns extras list for final_proj."""
        extras = []
        for t in vf_taps:
            psK = psA.tile([128, 128], FP32, tag="a")
            nc.tensor.matmul(psK[:, :], w9vf[:, 128 * t:128 * t + 128],
                             mefft[:], start=True, stop=True)
            kt = mpool.tile([128, 128], BF16, tag=f"kt{t}")
            nc.scalar.copy(kt[:], psK[:, :])
            extras.append((kt, xt, t))
        for t in yf_taps:
            mt = mpool.tile([128, 128], BF16, tag=f"mt{t}")
            nc.vector.tensor_scalar_mul(mt[:], p2t[:], dwyw[:, t:t + 1])
            extras.append((mt, yt, t))
        return extras

    def final_proj(b, mefft, p2t, vt, y2t, extras=()):
        out2d = io["out"][b].rearrange("c h w -> c (h w)")
        for nn in range(NCH_A):
            r0 = 2 * nn
            pf = psA.tile([128, 512], FP32, tag="a")
            nc.tensor.matmul(pf[:, :], mefft[:],
                             vt[:, 512 * nn:512 * nn + 512],
                             start=True, stop=False).annotate("mm_proj")
            nc.tensor.matmul(pf[:, :], p2t[:],
                             y2t[:, 512 * nn:512 * nn + 512],
                             start=False, stop=len(extras) == 0).annotate("mm_proj")
            for i, (wt, srct, t) in enumerate(extras):
                ti, tj = t // 3, t % 3
                sv = srct[:].rearrange("p (h w) -> p h w", h=HH)
                xv = sv[:, r0 + ti:r0 + ti + 2, tj:tj + 256]
                nc.tensor.matmul(pf[:, :], wt[:], xv, start=False,
                                 stop=i == len(extras) - 1).annotate("mm_projf")
            ot = opool.tile([128, 512], FP16, tag="ot")
            nc.scalar.copy(ot[:], pf[:, :]).annotate("cp_ot")
            dma(out2d[:, 512 * nn:512 * nn + 512], ot[:])

    # ================= schedule =================
    from collections import deque
    F = deque()
    with nc.allow_low_precision(reason="fp16 depthwise accumulation"):
        for b in range(B):
            if b == 0:
                xt, yt = xt0, yt0
            else:
                xt = xpool.tile([128, FREE], BF16, tag="x")
                yt = ypool.tile([128, FREE], BF16, tag="y")
                dma(yt[:], io["yh"][b].rearrange("c h w -> c (h w)"))
                dma(xt[:], io["xh"][b].rearrange("c h w -> c (h w)"))
            xts.append(xt)
            yts.append(yt)

            saT = sa_gate(b, yt)
            saTs.append(saT)
            vp = v1x1(b, xt)
            vsum, fvs = vsum_side(b, vp)
            vt, v_te, vf, fv = tap_fillers(V_TAPS[b], vp, dwvw, vpool, "vt")
            taps_te(b, xt, w9v, v_te, vt, merge=True)
            vts.append(vt)
            vfs.append(vf)
            y2t, y_te, yf, fy = tap_fillers(Y_TAPS[b], yt, dwyw, y2pool, "y2t")
            taps_te(b, yt, wyd, y_te, y2t, merge=True)
            y2ts.append(y2t)
            yfs.append(yf)
            F.extend(fvs)
            F.extend(fv)
            F.extend(fy)
            # a few fillers ahead of the QK stream
            for _ in range(3):
                if F:
                    F.popleft()()
            G, G2 = qk_gram(b, xt, saT, F)
            arst = stage_stats(b, G, G2, vsum)
            arres = issue_ar(b, arst)
            arreses.append(arres)
            if b == 1:
                mp0 = post_ar(0, arreses[0])

        ex0 = fold_weights(0, mp0[0], mp0[1], vfs[0], yfs[0], xts[0], yts[0])
        while F:
            F.popleft()()
        final_proj(0, mp0[0], mp0[1], vts[0], y2ts[0], ex0)
        mefft1, p2t1 = post_ar(1, arreses[1])
        ex1 = fold_weights(1, mefft1, p2t1, vfs[1], yfs[1], xts[1], yts[1])
        final_proj(1, mefft1, p2t1, vts[1], y2ts[1], ex1)

    ctx.close()


def build_nc():
    nc = bacc.Bacc("TRN2", target_bir_lowering=False, debug=False,
                   num_devices=NCORES)
    io = {}

    def inp(name, shape, dt):
        io[name] = nc.dram_tensor(name, shape, dt, kind="ExternalInput")

    inp("xh", [B, C, HH, WW], BF16)
    inp("yh", [B, C, HH, WW], BF16)
    inp("w9qk", [128, 9 * 256], BF16)
    inp("w9v", [128, 9 * 128], BF16)
    inp("w9vf", [128, 9 * 128], FP16)
    inp("wyd", [128, 9 * 128], BF16)
    inp("wv1x1", [128, 128], BF16)
    inp("saw1t", [128, 32], BF16)
    inp("w2rep", [128, 32], BF16)
    inp("w3rep", [128, 1], BF16)
    inp("spw1t", [128, 16], FP32)
    inp("spw2t", [16, 16], FP32)
    inp("spw3t", [16, 128], FP32)
    inp("projt", [128, 128], FP32)
    inp("dwvw", [128, 9], FP32)
    inp("dwyw", [128, 9], FP32)
    inp("consts", [128, 386], FP32)
    io["out"] = nc.dram_tensor("out", [B, C, RPC, W], FP16, kind="ExternalOutput")

    with tile.TileContext(nc) as tc:
        _emit(tc, io)
    nc.finalize()
    return nc


_CACHE = {}


def _prep_host(x, y, qkv_w, qkv_dw_w, proj_w, sa_w1, sa_w2, sa_w3,
               sp_w1, sp_w2, sp_w3, dw_w, temperature):
    import ml_dtypes
    bf = ml_dtypes.bfloat16
    f32 = np.float32

    x = np.asarray(x, f32)
    y = np.asarray(y, f32)
    xp = np.zeros((B, C, H + 2, W + 2), f32)
    xp[:, :, 1:H + 1, 1:W + 1] = x
    yp = np.zeros((B, C, H + 2, W + 2), f32)
    yp[:, :, 1:H + 1, 1:W + 1] = y
    xp = xp.astype(bf)
    yp = yp.astype(bf)

    qkv_w = np.asarray(qkv_w, f32)
    dw = np.asarray(qkv_dw_w, f32).reshape(3 * C, 9)
    w9qk = np.concatenate(
        [(qkv_w[:256] * dw[:256, t:t + 1]).T for t in range(9)], axis=1)
    w9v = np.concatenate(
        [(qkv_w[256:] * dw[256:, t:t + 1]).T for t in range(9)], axis=1)
    w9vf = np.concatenate(
        [(qkv_w[256:] * dw[256:, t:t + 1]) for t in range(9)], axis=1)
    dwy = np.asarray(dw_w, f32).reshape(C, 9)
    wyd = np.concatenate(
        [np.diag(dwy[:, t]) for t in range(9)], axis=1)

    w2rep = np.zeros((128, 32), f32)
    w3rep = np.zeros((128, 1), f32)
    for k in range(4):
        w2rep[32 * k:32 * k + 16, 0:16] = np.asarray(sa_w2, f32).T
        w3rep[32 * k:32 * k + 16] = np.asarray(sa_w3, f32).T
    saw1tp = np.zeros((128, 32), f32)
    saw1tp[:, 0:16] = np.asarray(sa_w1, f32).T

    consts = np.zeros((128, 386), f32)
    consts[:, 0:128] = np.eye(128, dtype=f32)
    ci = np.arange(128) // DH
    consts[:, 128:256] = (ci[:, None] == ci[None, :]).astype(f32)
    consts[:, 256] = np.asarray(temperature, f32).reshape(HD)[ci]
    consts[0, 257:385] = 1.0

    common = {
        "w9qk": w9qk.astype(bf), "w9v": w9v.astype(bf), "wyd": wyd.astype(bf),
        "w9vf": w9vf.astype(np.float16),
        "wv1x1": np.ascontiguousarray(qkv_w[256:].T).astype(bf),
        "saw1t": saw1tp.astype(bf),
        "w2rep": w2rep.astype(bf), "w3rep": w3rep.astype(bf),
        "spw1t": np.asarray(sp_w1, f32).T.copy(),
        "spw2t": np.asarray(sp_w2, f32).T.copy(),
        "spw3t": np.asarray(sp_w3, f32).T.copy(),
        "projt": np.asarray(proj_w, f32).T.copy(),
        "dwvw": np.ascontiguousarray(dw[256:]),
        "dwyw": np.ascontiguousarray(dwy),
        "consts": consts,
    }
    in_maps = []
    for i in range(NCORES):
        m = dict(common)
        m["xh"] = np.ascontiguousarray(xp[:, :, 32 * i:32 * i + HH, :])
        m["yh"] = np.ascontiguousarray(yp[:, :, 32 * i:32 * i + HH, :])
        in_maps.append(m)
    return in_maps


def kernel(**inputs):
    if "nc" not in _CACHE:
        _CACHE["nc"] = build_nc()
    nc = _CACHE["nc"]
    in_maps = _prep_host(**inputs)
    res = run_bass_kernel_spmd(nc, in_maps, core_ids=list(range(NCORES)))
    shards = [res.results[i]["out"] for i in range(NCORES)]
    return np.concatenate(shards, axis=2).astype(np.float32)


# revision 17
# speedup vs baseline: 2.9753x; 1.0319x over previous
"""Cross-Spatial-Attention Trainium2 kernel (8 NeuronCores, spatial sharding).

v2: engine-balanced. TensorE keeps the QK 9-tap fused conv (transposed
layout) + gram + SA gate + projections; the depthwise work for v and
dwconv(y) moves to the Vector/GpSimd engines as per-channel
multiply-accumulate passes (tensor_scalar 4x + tensor_tensor 2x, fp16),
fed by a cheap 1x1 conv for v_pre. The v-mean needed by the stats
AllReduce is computed from window sums of v_pre (row-sum side path) so
each batch's AllReduce fires right after its QK gram; batch1's
y-depthwise stays on TensorE as filler inside the AllReduce window.
"""

import numpy as np
from contextlib import ExitStack

import concourse.bass as bass
import concourse.bacc as bacc
import concourse.tile as tile
from concourse import mybir
from concourse.bass_utils import run_bass_kernel_spmd

FP32 = mybir.dt.float32
FP16 = mybir.dt.float16
BF16 = mybir.dt.bfloat16
AF = mybir.ActivationFunctionType
ALU = mybir.AluOpType

B, C, H, W = 2, 128, 256, 256
HD, DH = 8, 16
NCORES = 8
RPC = H // NCORES            # 32 rows per core
HH, WW = RPC + 2, W + 2      # 34 x 258 halo'd band
FREE = HH * WW               # 8772
NLOC = RPC * W               # 8192 output positions per band per batch
NCH_T = NLOC // 128          # 64 transposed chunks
NCH_A = NLOC // 512          # 16 layout-A chunks
NTOT = float(H * W)          # global spatial size

# tap engine assignment: per (tensor, batch) a list of 9 entries
# 'd' = DVE ts+tt, 'g' = DVE ts + GpSimd tt, 't' = TensorE dense fold
V_TAPS = {0: list("dddddddff"), 1: list("ddddddddd")}
Y_TAPS = {0: list("dddddddff"), 1: list("ddddddddd")}
FILL_EVERY = 3               # emit one DVE filler per this many QK chunks

TMPH = NLOC // 2             # DVE tap chunk
TMPG = NLOC // 4             # gpsimd tap chunk


def _emit(tc, io):
    nc = tc.nc
    ctx = ExitStack()

    wpool = ctx.enter_context(tc.tile_pool(name="wpool", bufs=1))
    xpool = ctx.enter_context(tc.tile_pool(name="xpool", bufs=2))
    ypool = ctx.enter_context(tc.tile_pool(name="ypool", bufs=2))
    vppool = ctx.enter_context(tc.tile_pool(name="vppool", bufs=2))
    vpool = ctx.enter_context(tc.tile_pool(name="vpool", bufs=2))
    y2pool = ctx.enter_context(tc.tile_pool(name="y2pool", bufs=2))
    tmppool = ctx.enter_context(tc.tile_pool(name="tmppool", bufs=1))
    spool = ctx.enter_context(tc.tile_pool(name="spool", bufs=1))
    rpool = ctx.enter_context(tc.tile_pool(name="rpool", bufs=4))
    mpool = ctx.enter_context(tc.tile_pool(name="mpool", bufs=1))
    opool = ctx.enter_context(tc.tile_pool(name="opool", bufs=2))
    psA = ctx.enter_context(tc.tile_pool(name="psA", bufs=2, space="PSUM"))
    psQK = ctx.enter_context(tc.tile_pool(name="psQK", bufs=4, space="PSUM"))
    psG = ctx.enter_context(tc.tile_pool(name="psG", bufs=1, space="PSUM"))
    dpool = ctx.enter_context(tc.tile_pool(name="dram", bufs=4, space="DRAM"))

    def dma(dst, src):
        nc.sync.dma_start(out=dst, in_=src)

    def wload(name, shape, dt=BF16):
        t = wpool.tile(shape, dt, tag=name)
        dma(t[:], io[name][:])
        return t

    saw1t = wload("saw1t", [128, 32])
    w2rep = wload("w2rep", [128, 32])
    w3rep = wload("w3rep", [128, 1])
    xt0 = xpool.tile([128, FREE], BF16, tag="x")
    yt0 = ypool.tile([128, FREE], BF16, tag="y")
    yh0 = io["yh"][0].rearrange("c h w -> c (h w)")
    xh0 = io["xh"][0].rearrange("c h w -> c (h w)")
    for r0, r1 in ((0, 11), (11, 22), (22, 34)):
        dma(yt0[:, r0 * WW:r1 * WW], yh0[:, r0 * WW:r1 * WW])
    for r0, r1 in ((0, 11), (11, 22), (22, 34)):
        dma(xt0[:, r0 * WW:r1 * WW], xh0[:, r0 * WW:r1 * WW])
    w9qk = wload("w9qk", [128, 9 * 256])     # tap t at cols [256t:256t+256]
    wv1x1 = wload("wv1x1", [128, 128])       # v 1x1: [ic, oc]
    dwvw = wload("dwvw", [128, 9], FP32)     # v depthwise tap weights
    dwyw = wload("dwyw", [128, 9], FP32)     # y depthwise tap weights
    consts = wload("consts", [128, 386], FP32)
    need_w9v = any(s == "t" for b in range(B) for s in V_TAPS[b])
    w9v = wload("w9v", [128, 9 * 128]) if need_w9v else None
    need_w9vf = any(s == "f" for b in range(B) for s in V_TAPS[b])
    w9vf = wload("w9vf", [128, 9 * 128], FP16) if need_w9vf else None
    need_wyd = any(s == "t" for b in range(B) for s in Y_TAPS[b])
    wyd = wload("wyd", [128, 9 * 128]) if need_wyd else None
    spw1t = wload("spw1t", [128, 16], FP32)
    spw2t = wload("spw2t", [16, 16], FP32)
    spw3t = wload("spw3t", [16, 128], FP32)
    projt = wload("projt", [128, 128], FP32)
    eye = consts[:, 0:128]
    bdmask = consts[:, 128:256]
    tempp = consts[:, 256:257]
    onesrow = consts[0:1, 257:385]

    xts, yts, vts, y2ts, saTs = [], [], [], [], []
    vfs, yfs = [], []
    arreses = []

    # ---------------- helpers ----------------
    def sa_gate(b, yt):
        """spatial-attention gate -> saT [128, 64] (col j = chunk j)"""
        s1 = spool.tile([128, 2048], BF16, tag="s1")
        s2 = s1
        for g in range(4):
            ps1 = psA.tile([128, 512], FP32, tag="a")
            for k in range(4):
                nn = 4 * g + k
                r0 = 2 * nn
                yv = yt[:].rearrange("p (h w) -> p h w", h=HH)[
                    :, r0 + 1:r0 + 3, 1:257]
                nc.tensor.matmul(ps1[32 * k:32 * k + 32, :], saw1t[:, :], yv,
                                 start=True, stop=True,
                                 tile_position=(0, 32 * k)).annotate("mm_sa")
            if g % 2 == 0:
                nc.vector.tensor_scalar_max(s1[:, 512 * g:512 * g + 512], ps1[:, :], 0.0)
            else:
                nc.scalar.activation(s1[:, 512 * g:512 * g + 512], ps1[:, :], AF.Relu)
        for g in range(4):
            ps2 = psA.tile([128, 512], FP32, tag="a")
            for k in range(4):
                nc.tensor.matmul(ps2[32 * k:32 * k + 32, :],
                                 w2rep[32 * k:32 * k + 16, :],
                                 s1[32 * k:32 * k + 16, 512 * g:512 * g + 512],
                                 start=True, stop=True,
                                 tile_position=(32 * k, 32 * k))
            if g % 2 == 0:
                nc.vector.tensor_scalar_max(s2[:, 512 * g:512 * g + 512], ps2[:, :], 0.0)
            else:
                nc.scalar.activation(s2[:, 512 * g:512 * g + 512], ps2[:, :], AF.Relu)
        saT_ps = psQK.tile([128, 64], FP32, tag="qk")
        for j in range(NCH_T):
            nn, off = j // 4, (j % 4) * 128
            g, k = nn // 4, nn % 4
            nc.tensor.matmul(saT_ps[:, j:j + 1],
                             s2[32 * k:32 * k + 16,
                                512 * g + off:512 * g + off + 128],
                             w3rep[32 * k:32 * k + 16, :],
                             start=True, stop=True, tile_position=(32 * k, 0))
        saT = mpool.tile([128, 64], FP32, tag="saT")
        nc.scalar.activation(saT[:], saT_ps[:], AF.Sigmoid)
        return saT

    def v1x1(b, xt):
        """v_pre = Wv @ x over the halo'd band -> [128, FREE] fp16"""
        vp = vppool.tile([128, FREE], FP16, tag="vp")
        c0 = 0
        while c0 < FREE:
            w = min(512, FREE - c0)
            pv = psA.tile([128, 512], FP32, tag="a")
            nc.tensor.matmul(pv[:, 0:w], wv1x1[:, :], xt[:, c0:c0 + w],
                             start=True, stop=True).annotate("mm_v1x1")
            nc.scalar.copy(vp[:, c0:c0 + w], pv[:, 0:w]).annotate("cp_vp")
            c0 += w
        return vp

    def vsum_side(b, vp):
        """vsum[c] = sum over band of v (exact, via window sums of v_pre).

        Returns (vsum tile, filler closures). The closures do the actual
        work (ACT row sums + small DVE combines) and must be popped before
        stage_stats(b) runs."""
        vv = vp[:].rearrange("p (h w) -> p h w", h=HH)

        def edge(k):
            return vv[:, :, k:k + 1].rearrange("p h w -> p (h w)")

        fr = mpool.tile([128, 34], FP32, tag=f"fr{b}")
        junkr = mpool.tile([128, 258], FP16, tag="junkr")
        vsum = mpool.tile([128, 1], FP32, tag=f"vsum{b}")
        closures = []

        def rows(r0, r1):
            def f():
                for r in range(r0, r1):
                    rowap = vv[:, r:r + 1, :].rearrange("p h w -> p (h w)")
                    nc.scalar.activation(junkr[:], rowap, AF.Copy,
                                         accum_out=fr[:, r:r + 1]).annotate("vsum_red")
            return f

        for r0 in range(0, HH, 6):
            closures.append(rows(r0, min(r0 + 6, HH)))

        def mini():
            # rs block tj at cols [34*tj:34*tj+34]: row sums over tj..tj+255
            rs = mpool.tile([128, 102], FP32, tag="rs")
            pairs = [(256, 257), (0, 257), (0, 1)]
            for tj, (ka, kb) in enumerate(pairs):
                nc.vector.tensor_tensor(rs[:, 34 * tj:34 * tj + 34], fr[:],
                                        edge(ka), ALU.subtract)
                nc.vector.tensor_tensor(rs[:, 34 * tj:34 * tj + 34],
                                        rs[:, 34 * tj:34 * tj + 34],
                                        edge(kb), ALU.subtract)
            rs3 = rs[:].rearrange("p (t r) -> p t r", t=3)
            tj_tot = mpool.tile([128, 3], FP32, tag="tjt")
            nc.vector.tensor_reduce(tj_tot[:], rs3, mybir.AxisListType.X, ALU.add)
            ex = [(32, 33), (0, 33), (0, 1)]
            ws = mpool.tile([128, 9], FP32, tag="ws")
            for ti in range(3):
                a_, b_ = ex[ti]
                ra = rs3[:, :, a_:a_ + 1].rearrange("p t r -> p (t r)")
                rb = rs3[:, :, b_:b_ + 1].rearrange("p t r -> p (t r)")
                nc.vector.tensor_tensor(ws[:, 3 * ti:3 * ti + 3], tj_tot[:],
                                        ra, ALU.subtract)
                nc.vector.tensor_tensor(ws[:, 3 * ti:3 * ti + 3],
                                        ws[:, 3 * ti:3 * ti + 3],
                                        rb, ALU.subtract)
            wsw = mpool.tile([128, 9], FP32, tag="wsw")
            nc.vector.tensor_tensor(wsw[:], ws[:], dwvw[:], ALU.mult)
            nc.vector.tensor_reduce(vsum[:], wsw[:], mybir.AxisListType.X,
                                    ALU.add)

        closures.append(mini)
        return vsum, closures

    def qk_gram(b, xt, saT, fillers):
        """QK 9-tap fused conv in transposed layout + gram accumulation."""
        G = psG.tile([128, 256], FP32, tag="G")
        G2 = psG.tile([128, 128], FP32, tag="G2")
        for j in range(NCH_T):
            if j % FILL_EVERY == FILL_EVERY - 1 and fillers:
                fillers.popleft()()
            r, c0 = j // 2, (j % 2) * 128
            pqk = psQK.tile([128, 256], FP32, tag="qk")
            for t in range(9):
                ti, tj = t // 3, t % 3
                base = (r + ti) * WW + c0 + tj
                nc.tensor.matmul(pqk[:, :], xt[:, base:base + 128],
                                 w9qk[:, 256 * t:256 * t + 256],
                                 start=(t == 0), stop=(t == 8)).annotate("mm_qk")
            rt = rpool.tile([128, 256], BF16, tag="ring")
            nc.vector.tensor_scalar_mul(rt[:, 0:128], pqk[:, 0:128],
                                        saT[:, j:j + 1]).annotate("cp_rtq")
            nc.scalar.copy(rt[:, 128:256], pqk[:, 128:256]).annotate("cp_rtk")
            nc.tensor.matmul(G[:, 0:256], rt[:, 0:128], rt[:, 0:256],
                             start=(j == 0), stop=(j == NCH_T - 1),
                             skip_group_check=True).annotate("mm_gram")
            nc.tensor.matmul(G2[:, :], rt[:, 128:256], rt[:, 128:256],
                             start=(j == 0), stop=(j == NCH_T - 1),
                             skip_group_check=True).annotate("mm_gram")
        return G, G2

    def stage_stats(b, G, G2, vsum):
        """arst [128, 131]: [Gqk | qd | kd | vsum]"""
        arst = mpool.tile([128, 131], FP32, tag=f"arst{b}")
        junk = mpool.tile([128, 128], FP32, tag="junk")
        nc.vector.tensor_copy(arst[:, 0:128], G[:, 128:256])
        nc.vector.scalar_tensor_tensor(junk[:], G[:, 0:128], 1.0, eye,
                                       ALU.mult, ALU.mult,
                                       accum_out=arst[:, 128:129])
        nc.vector.scalar_tensor_tensor(junk[:], G2[:, :], 1.0, eye,
                                       ALU.mult, ALU.mult,
                                       accum_out=arst[:, 129:130])
        nc.vector.tensor_copy(arst[:, 130:131], vsum[:])
        return arst

    def issue_ar(b, arst):
        din = dpool.tile([128, 131], FP32, tag=f"din{b}")
        dout = dpool.tile([128, 131], FP32, tag=f"dout{b}")
        dma(din[:], arst[:])
        nc.gpsimd.collective_compute(
            "AllReduce", ALU.add,
            replica_groups=[list(range(NCORES))],
            ins=[din[:].opt()], outs=[dout[:].opt()])
        arres = mpool.tile([128, 131], FP32, tag=f"arres{b}")
        dma(arres[:], dout[:])
        return arres

    def tap_fillers(spec, src, dwv, acc_pool, acc_tag):
        """depthwise 3x3 over halo'd src -> acc [128, NLOC] fp16.

        Returns (acc, te_taps, fillers): each filler is a closure emitting
        one half-width tap op pair; pop them in order."""
        sv = src[:].rearrange("p (h w) -> p h w", h=HH)
        acc = acc_pool.tile([128, NLOC], FP16, tag=acc_tag)
        d_taps = [t for t in range(9) if spec[t] == "d"]
        g_taps = [t for t in range(9) if spec[t] == "g"]
        te_taps = [t for t in range(9) if spec[t] == "t"]
        f_taps = [t for t in range(9) if spec[t] == "f"]

        def shifted(t, c0, w):
            ti, tj = t // 3, t % 3
            r0, cw = c0 // W, c0 % W
            nr = w // W
            return sv[:, ti + r0:ti + r0 + nr, tj + cw:tj + cw + W]

        fillers = []
        for i, t in enumerate(d_taps):
            for h in range(NLOC // TMPH):
                def f(t=t, h=h, first=(i == 0)):
                    c0 = h * TMPH
                    dst = acc[:, c0:c0 + TMPH]
                    dstv = dst.rearrange("p (h w) -> p h w", h=TMPH // W)
                    if first:
                        nc.vector.tensor_scalar_mul(
                            dstv, shifted(t, c0, TMPH),
                            dwv[:, t:t + 1]).annotate("tap_ts")
                    else:
                        tmp = tmppool.tile([128, TMPH], FP16, tag="tmp")
                        tmpv = tmp[:].rearrange("p (h w) -> p h w", h=TMPH // W)
                        nc.vector.tensor_scalar_mul(
                            tmpv, shifted(t, c0, TMPH),
                            dwv[:, t:t + 1]).annotate("tap_ts")
                        nc.vector.tensor_tensor(dst, dst, tmp[:],
                                                ALU.add).annotate("tap_tt")
                fillers.append(f)
        for t in g_taps:
            for h in range(NLOC // TMPH):
                def f(t=t, h=h):
                    for q in range(TMPH // TMPG):
                        c0 = h * TMPH + q * TMPG
                        dst = acc[:, c0:c0 + TMPG]
                        tmpg = tmppool.tile([128, TMPG], FP16, tag="tmpg")
                        tmpgv = tmpg[:].rearrange("p (h w) -> p h w",
                                                  h=TMPG // W)
                        nc.vector.tensor_scalar_mul(
                            tmpgv, shifted(t, c0, TMPG),
                            dwv[:, t:t + 1]).annotate("tap_gts")
                        nc.gpsimd.tensor_tensor(dst, dst, tmpg[:],
                                                ALU.add).annotate("tap_gtt")
                fillers.append(f)
        return acc, te_taps, f_taps, fillers

    def taps_te(b, src, w9, te_taps, acc, merge):
        """TE dense-fold taps over halo'd src, baseline-style; merge into acc."""
        if not te_taps:
            return
        sv = src[:].rearrange("p (h w) -> p h w", h=HH)
        for nn in range(NCH_A):
            r0 = 2 * nn
            py = psA.tile([128, 512], FP32, tag="a")
            for i, t in enumerate(te_taps):
                ti, tj = t // 3, t % 3
                xv = sv[:, r0 + ti:r0 + ti + 2, tj:tj + 256]
                nc.tensor.matmul(py[:, :], w9[:, 128 * t:128 * t + 128], xv,
                                 start=(i == 0),
                                 stop=(i == len(te_taps) - 1)).annotate("mm_yte")
            if merge:
                nc.vector.tensor_tensor(acc[:, 512 * nn:512 * nn + 512],
                                        acc[:, 512 * nn:512 * nn + 512],
                                        py[:, :], ALU.add).annotate("cp_te")
            else:
                nc.scalar.copy(acc[:, 512 * nn:512 * nn + 512],
                               py[:, :]).annotate("cp_te")

    def post_ar(b, arres):
        """norms -> softmax -> Meff/p2t; returns (mefft fp16, p2t fp16)"""
        rqk = mpool.tile([128, 2], FP32, tag="rqk")
        srt = mpool.tile([128, 2], FP32, tag="srt")
        dcat = arres[:, 128:130]
        nc.scalar.activation(srt[:], dcat, AF.Sqrt)
        nc.vector.tensor_scalar_max(srt[:], srt[:], 1e-12)
        nc.vector.reciprocal(rqk[:], srt[:])
        r2 = mpool.tile([128, 2], FP32, tag="r2")
        nc.vector.tensor_tensor(r2[:], rqk[:], rqk[:], ALU.mult)
        nc.vector.tensor_tensor(r2[:], r2[:], dcat, ALU.mult)
        nc.vector.tensor_scalar(r2[:], r2[:], -0.5, 1.5, ALU.mult, ALU.add)
        nc.vector.tensor_tensor(rqk[:], rqk[:], r2[:], ALU.mult)
        rqt = mpool.tile([128, 1], FP32, tag="rqt")
        nc.vector.tensor_tensor(rqt[:], rqk[:, 0:1], tempp, ALU.mult)

        ps1 = psA.tile([128, 128], FP32, tag="a")
        nc.tensor.matmul(ps1[0:1, :], rqk[:, 1:2], eye, start=True, stop=True)
        rkrow = mpool.tile([1, 128], FP32, tag="rkrow")
        nc.scalar.copy(rkrow[:], ps1[0:1, :])
        ps2 = psA.tile([128, 128], FP32, tag="a")
        nc.tensor.matmul(ps2[:, :], onesrow, rkrow[:], start=True, stop=True)

        gh = mpool.tile([128, 128], FP32, tag="gh")
        nc.vector.scalar_tensor_tensor(gh[:], arres[:, 0:128], rqt[:, 0:1],
                                       ps2[:, :], ALU.mult, ALU.mult)
        sm = mpool.tile([128, 128], FP32, tag="sm")
        nc.scalar.activation(sm[:], gh[:], AF.Exp)
        rs_ = mpool.tile([128, 1], FP32, tag="rssm")
        nc.vector.scalar_tensor_tensor(sm[:], sm[:], 1.0, bdmask,
                                       ALU.mult, ALU.mult, accum_out=rs_[:])
        nc.vector.reciprocal(rs_[:], rs_[:])
        attn = mpool.tile([128, 128], FP32, tag="attn")
        nc.vector.tensor_scalar_mul(attn[:], sm[:], rs_[:, 0:1])

        psM = psA.tile([128, 128], FP32, tag="a")
        nc.tensor.matmul(psM[:, :], attn[:], projt[:], start=True, stop=True)
        mefft = mpool.tile([128, 128], FP16, tag="mefft")
        nc.scalar.copy(mefft[:], psM[:, :])

        psT = psA.tile([128, 128], FP32, tag="a")
        nc.tensor.transpose(psT[:, :], attn[:], eye)
        attnt = mpool.tile([128, 128], FP32, tag="attnt")
        nc.vector.tensor_copy(attnt[:], psT[:, :])
        psP = psA.tile([128, 1], FP32, tag="a")
        nc.tensor.matmul(psP[:, :], attnt[:], arres[:, 130:131],
                         start=True, stop=True)
        pooled = mpool.tile([128, 1], FP32, tag="pooled")
        nc.scalar.activation(pooled[:], psP[:, :], AF.Copy, scale=1.0 / NTOT)

        psg1 = psA.tile([16, 1], FP32, tag="a")
        nc.tensor.matmul(psg1[:, :], spw1t[:], pooled[:], start=True, stop=True)
        g1 = mpool.tile([16, 1], FP32, tag="g1")
        nc.scalar.activation(g1[:], psg1[:, :], AF.Gelu)
        psg2 = psA.tile([16, 1], FP32, tag="a")
        nc.tensor.matmul(psg2[:, :], spw2t[:], g1[:], start=True, stop=True)
        g2 = mpool.tile([16, 1], FP32, tag="g2")
        nc.scalar.activation(g2[:], psg2[:, :], AF.Gelu)
        psg3 = psA.tile([128, 1], FP32, tag="a")
        nc.tensor.matmul(psg3[:, :], spw3t[:], g2[:], start=True, stop=True)
        spec = mpool.tile([128, 1], FP32, tag="spec")
        nc.scalar.activation(spec[:], psg3[:, :], AF.Sigmoid)

        p2t = mpool.tile([128, 128], FP16, tag="p2t")
        nc.vector.tensor_scalar_mul(p2t[:], projt[:], spec[:, 0:1])
        return mefft, p2t

    def fold_weights(b, mefft, p2t, vf_taps, yf_taps, xt, yt):
        """lhsT weights for proj-folded taps: K_t = w9vf_t^T @ mefft (v),
        M_t = p2t * dwy_t (y). Returns extras list for final_proj."""
        extras = []
        for t in vf_taps:
            psK = psA.tile([128, 128], FP32, tag="a")
            nc.tensor.matmul(psK[:, :], w9vf[:, 128 * t:128 * t + 128],
                             mefft[:], start=True, stop=True)
            kt = mpool.tile([128, 128], BF16, tag=f"kt{t}")
            nc.scalar.copy(kt[:], psK[:, :])
            extras.append((kt, xt, t))
        for t in yf_taps:
            mt = mpool.tile([128, 128], BF16, tag=f"mt{t}")
            nc.vector.tensor_scalar_mul(mt[:], p2t[:], dwyw[:, t:t + 1])
            extras.append((mt, yt, t))
        return extras

    def final_proj(b, mefft, p2t, vt, y2t, extras=()):
        out2d = io["out"][b].rearrange("c h w -> c (h w)")
        for nn in range(NCH_A):
            r0 = 2 * nn
            pf = psA.tile([128, 512], FP32, tag="a")
            nc.tensor.matmul(pf[:, :], mefft[:],
                             vt[:, 512 * nn:512 * nn + 512],
                             start=True, stop=False).annotate("mm_proj")
            nc.tensor.matmul(pf[:, :], p2t[:],
                             y2t[:, 512 * nn:512 * nn + 512],
                             start=False, stop=len(extras) == 0).annotate("mm_proj")
            for i, (wt, srct, t) in enumerate(extras):
                ti, tj = t // 3, t % 3
                sv = srct[:].rearrange("p (h w) -> p h w", h=HH)
                xv = sv[:, r0 + ti:r0 + ti + 2, tj:tj + 256]
                nc.tensor.matmul(pf[:, :], wt[:], xv, start=False,
                                 stop=i == len(extras) - 1).annotate("mm_projf")
            ot = opool.tile([128, 512], FP16, tag="ot")
            nc.scalar.copy(ot[:], pf[:, :]).annotate("cp_ot")
            dma(out2d[:, 512 * nn:512 * nn + 512], ot[:])

    # ================= schedule =================
    from collections import deque
    F = deque()
    with nc.allow_low_precision(reason="fp16 depthwise accumulation"):
        for b in range(B):
            if b == 0:
                xt, yt = xt0, yt0
            else:
                xt = xpool.tile([128, FREE], BF16, tag="x")
                yt = ypool.tile([128, FREE], BF16, tag="y")
                dma(yt[:], io["yh"][b].rearrange("c h w -> c (h w)"))
                dma(xt[:], io["xh"][b].rearrange("c h w -> c (h w)"))
            xts.append(xt)
            yts.append(yt)

            saT = sa_gate(b, yt)
            saTs.append(saT)
            vp = v1x1(b, xt)
            vsum, fvs = vsum_side(b, vp)
            vt, v_te, vf, fv = tap_fillers(V_TAPS[b], vp, dwvw, vpool, "vt")
            taps_te(b, xt, w9v, v_te, vt, merge=True)
            vts.append(vt)
            vfs.append(vf)
            y2t, y_te, yf, fy = tap_fillers(Y_TAPS[b], yt, dwyw, y2pool, "y2t")
            taps_te(b, yt, wyd, y_te, y2t, merge=True)
            y2ts.append(y2t)
            yfs.append(yf)
            F.extend(fvs)
            F.extend(fv)
            F.extend(fy)
            # a few fillers ahead of the QK stream
            for _ in range(3):
                if F:
                    F.popleft()()
            G, G2 = qk_gram(b, xt, saT, F)
            arst = stage_stats(b, G, G2, vsum)
            arres = issue_ar(b, arst)
            arreses.append(arres)
            if b == 1:
                mp0 = post_ar(0, arreses[0])

        ex0 = fold_weights(0, mp0[0], mp0[1], vfs[0], yfs[0], xts[0], yts[0])
        while F:
            F.popleft()()
        final_proj(0, mp0[0], mp0[1], vts[0], y2ts[0], ex0)
        mefft1, p2t1 = post_ar(1, arreses[1])
        ex1 = fold_weights(1, mefft1, p2t1, vfs[1], yfs[1], xts[1], yts[1])
        final_proj(1, mefft1, p2t1, vts[1], y2ts[1], ex1)

    ctx.close()


def build_nc():
    nc = bacc.Bacc("TRN2", target_bir_lowering=False, debug=False,
                   num_devices=NCORES)
    io = {}

    def inp(name, shape, dt):
        io[name] = nc.dram_tensor(name, shape, dt, kind="ExternalInput")

    inp("xh", [B, C, HH, WW], BF16)
    inp("yh", [B, C, HH, WW], BF16)
    inp("w9qk", [128, 9 * 256], BF16)
    inp("w9v", [128, 9 * 128], BF16)
    inp("w9vf", [128, 9 * 128], FP16)
    inp("wyd", [128, 9 * 128], BF16)
    inp("wv1x1", [128, 128], BF16)
    inp("saw1t", [128, 32], BF16)
    inp("w2rep", [128, 32], BF16)
    inp("w3rep", [128, 1], BF16)
    inp("spw1t", [128, 16], FP32)
    inp("spw2t", [16, 16], FP32)
    inp("spw3t", [16, 128], FP32)
    inp("projt", [128, 128], FP32)
    inp("dwvw", [128, 9], FP32)
    inp("dwyw", [128, 9], FP32)
    inp("consts", [128, 386], FP32)
    io["out"] = nc.dram_tensor("out", [B, C, RPC, W], FP16, kind="ExternalOutput")

    with tile.TileContext(nc) as tc:
        _emit(tc, io)
    nc.finalize()
    return nc


_CACHE = {}


def _prep_host(x, y, qkv_w, qkv_dw_w, proj_w, sa_w1, sa_w2, sa_w3,
               sp_w1, sp_w2, sp_w3, dw_w, temperature):
    import ml_dtypes
    bf = ml_dtypes.bfloat16
    f32 = np.float32

    x = np.asarray(x, f32)
    y = np.asarray(y, f32)
    xp = np.zeros((B, C, H + 2, W + 2), f32)
    xp[:, :, 1:H + 1, 1:W + 1] = x
    yp = np.zeros((B, C, H + 2, W + 2), f32)
    yp[:, :, 1:H + 1, 1:W + 1] = y
    xp = xp.astype(bf)
    yp = yp.astype(bf)

    qkv_w = np.asarray(qkv_w, f32)
    dw = np.asarray(qkv_dw_w, f32).reshape(3 * C, 9)
    w9qk = np.concatenate(
        [(qkv_w[:256] * dw[:256, t:t + 1]).T for t in range(9)], axis=1)
    w9v = np.concatenate(
        [(qkv_w[256:] * dw[256:, t:t + 1]).T for t in range(9)], axis=1)
    w9vf = np.concatenate(
        [(qkv_w[256:] * dw[256:, t:t + 1]) for t in range(9)], axis=1)
    dwy = np.asarray(dw_w, f32).reshape(C, 9)
    wyd = np.concatenate(
        [np.diag(dwy[:, t]) for t in range(9)], axis=1)

    w2rep = np.zeros((128, 32), f32)
    w3rep = np.zeros((128, 1), f32)
    for k in range(4):
        w2rep[32 * k:32 * k + 16, 0:16] = np.asarray(sa_w2, f32).T
        w3rep[32 * k:32 * k + 16] = np.asarray(sa_w3, f32).T
    saw1tp = np.zeros((128, 32), f32)
    saw1tp[:, 0:16] = np.asarray(sa_w1, f32).T

    consts = np.zeros((128, 386), f32)
    consts[:, 0:128] = np.eye(128, dtype=f32)
    ci = np.arange(128) // DH
    consts[:, 128:256] = (ci[:, None] == ci[None, :]).astype(f32)
    consts[:, 256] = np.asarray(temperature, f32).reshape(HD)[ci]
    consts[0, 257:385] = 1.0

    common = {
        "w9qk": w9qk.astype(bf), "w9v": w9v.astype(bf), "wyd": wyd.astype(bf),
        "w9vf": w9vf.astype(np.float16),
        "wv1x1": np.ascontiguousarray(qkv_w[256:].T).astype(bf),
        "saw1t": saw1tp.astype(bf),
        "w2rep": w2rep.astype(bf), "w3rep": w3rep.astype(bf),
        "spw1t": np.asarray(sp_w1, f32).T.copy(),
        "spw2t": np.asarray(sp_w2, f32).T.copy(),
        "spw3t": np.asarray(sp_w3, f32).T.copy(),
        "projt": np.asarray(proj_w, f32).T.copy(),
        "dwvw": np.ascontiguousarray(dw[256:]),
        "dwyw": np.ascontiguousarray(dwy),
        "consts": consts,
    }
    in_maps = []
    for i in range(NCORES):
        m = dict(common)
        m["xh"] = np.ascontiguousarray(xp[:, :, 32 * i:32 * i + HH, :])
        m["yh"] = np.ascontiguousarray(yp[:, :, 32 * i:32 * i + HH, :])
        in_maps.append(m)
    return in_maps


def kernel(**inputs):
    if "nc" not in _CACHE:
        _CACHE["nc"] = build_nc()
    nc = _CACHE["nc"]
    in_maps = _prep_host(**inputs)
    res = run_bass_kernel_spmd(nc, in_maps, core_ids=list(range(NCORES)))
    shards = [res.results[i]["out"] for i in range(NCORES)]
    return np.concatenate(shards, axis=2).astype(np.float32)


# revision 20
# speedup vs baseline: 3.1298x; 1.0519x over previous
"""Cross-Spatial-Attention Trainium2 kernel (8 NeuronCores, spatial sharding).

v2: engine-balanced. TensorE keeps the QK 9-tap fused conv (transposed
layout) + gram + SA gate + projections; the depthwise work for v and
dwconv(y) moves to the Vector/GpSimd engines as per-channel
multiply-accumulate passes (tensor_scalar 4x + tensor_tensor 2x, fp16),
fed by a cheap 1x1 conv for v_pre. The v-mean needed by the stats
AllReduce is computed from window sums of v_pre (row-sum side path) so
each batch's AllReduce fires right after its QK gram; batch1's
y-depthwise stays on TensorE as filler inside the AllReduce window.
"""

import numpy as np
from contextlib import ExitStack

import concourse.bass as bass
import concourse.bacc as bacc
import concourse.tile as tile
from concourse import mybir
from concourse.bass_utils import run_bass_kernel_spmd

FP32 = mybir.dt.float32
FP16 = mybir.dt.float16
BF16 = mybir.dt.bfloat16
AF = mybir.ActivationFunctionType
ALU = mybir.AluOpType

B, C, H, W = 2, 128, 256, 256
HD, DH = 8, 16
NCORES = 8
RPC = H // NCORES            # 32 rows per core
HH, WW = RPC + 2, W + 2      # 34 x 258 halo'd band
FREE = HH * WW               # 8772
NLOC = RPC * W               # 8192 output positions per band per batch
NCH_T = NLOC // 128          # 64 transposed chunks
NCH_A = NLOC // 512          # 16 layout-A chunks
NTOT = float(H * W)          # global spatial size

# tap engine assignment: per (tensor, batch) a list of 9 entries
# 'd' = DVE ts+tt, 'g' = DVE ts + GpSimd tt, 't' = TensorE dense fold
V_TAPS = {0: list("dddddddff"), 1: list("ddddddddd")}
Y_TAPS = {0: list("dddddddff"), 1: list("ddddddddd")}
FILL_EVERY = 3               # emit one DVE filler per this many QK chunks

TMPH = NLOC // 2             # DVE tap chunk
TMPG = NLOC // 4             # gpsimd tap chunk


def _emit(tc, io):
    nc = tc.nc
    ctx = ExitStack()

    wpool = ctx.enter_context(tc.tile_pool(name="wpool", bufs=1))
    xpool = ctx.enter_context(tc.tile_pool(name="xpool", bufs=2))
    ypool = ctx.enter_context(tc.tile_pool(name="ypool", bufs=2))
    vppool = ctx.enter_context(tc.tile_pool(name="vppool", bufs=2))
    vpool = ctx.enter_context(tc.tile_pool(name="vpool", bufs=2))
    y2pool = ctx.enter_context(tc.tile_pool(name="y2pool", bufs=2))
    tmppool = ctx.enter_context(tc.tile_pool(name="tmppool", bufs=1))
    spool = ctx.enter_context(tc.tile_pool(name="spool", bufs=1))
    rpool = ctx.enter_context(tc.tile_pool(name="rpool", bufs=4))
    mpool = ctx.enter_context(tc.tile_pool(name="mpool", bufs=1))
    opool = ctx.enter_context(tc.tile_pool(name="opool", bufs=2))
    psA = ctx.enter_context(tc.tile_pool(name="psA", bufs=2, space="PSUM"))
    psQK = ctx.enter_context(tc.tile_pool(name="psQK", bufs=4, space="PSUM"))
    psG = ctx.enter_context(tc.tile_pool(name="psG", bufs=1, space="PSUM"))
    dpool = ctx.enter_context(tc.tile_pool(name="dram", bufs=4, space="DRAM"))

    def dma(dst, src):
        nc.sync.dma_start(out=dst, in_=src)

    def wload(name, shape, dt=BF16):
        t = wpool.tile(shape, dt, tag=name)
        dma(t[:], io[name][:])
        return t

    saw1t = wload("saw1t", [128, 32])
    w2rep = wload("w2rep", [128, 32])
    w3rep = wload("w3rep", [128, 1])
    xt0 = xpool.tile([128, FREE], BF16, tag="x")
    yt0 = ypool.tile([128, FREE], BF16, tag="y")
    yh0 = io["yh"][0].rearrange("c h w -> c (h w)")
    xh0 = io["xh"][0].rearrange("c h w -> c (h w)")
    for r0, r1 in ((0, 11), (11, 22), (22, 34)):
        dma(yt0[:, r0 * WW:r1 * WW], yh0[:, r0 * WW:r1 * WW])
    for r0, r1 in ((0, 11), (11, 22), (22, 34)):
        dma(xt0[:, r0 * WW:r1 * WW], xh0[:, r0 * WW:r1 * WW])
    w9qk = wload("w9qk", [128, 9 * 256])     # tap t at cols [256t:256t+256]
    wv1x1 = wload("wv1x1", [128, 128])       # v 1x1: [ic, oc]
    dwvw = wload("dwvw", [128, 9], FP32)     # v depthwise tap weights
    dwyw = wload("dwyw", [128, 9], FP32)     # y depthwise tap weights
    consts = wload("consts", [128, 386], FP32)
    need_w9v = any(s == "t" for b in range(B) for s in V_TAPS[b])
    w9v = wload("w9v", [128, 9 * 128]) if need_w9v else None
    need_w9vf = any(s == "f" for b in range(B) for s in V_TAPS[b])
    w9vf = wload("w9vf", [128, 9 * 128], FP16) if need_w9vf else None
    need_wyd = any(s == "t" for b in range(B) for s in Y_TAPS[b])
    wyd = wload("wyd", [128, 9 * 128]) if need_wyd else None
    spw1t = wload("spw1t", [128, 16], FP32)
    spw2t = wload("spw2t", [16, 16], FP32)
    spw3t = wload("spw3t", [16, 128], FP32)
    projt = wload("projt", [128, 128], FP32)
    eye = consts[:, 0:128]
    bdmask = consts[:, 128:256]
    tempp = consts[:, 256:257]
    onesrow = consts[0:1, 257:385]

    xts, yts, vts, y2ts, saTs = [], [], [], [], []
    vfs, yfs = [], []
    arreses = []

    # ---------------- helpers ----------------
    def sa_gate(b, yt):
        """spatial-attention gate -> saT [128, 64] (col j = chunk j)"""
        s1 = spool.tile([128, 2048], BF16, tag="s1")
        s2 = s1
        for g in range(4):
            ps1 = psA.tile([128, 512], FP32, tag="a")
            for k in range(4):
                nn = 4 * g + k
                r0 = 2 * nn
                yv = yt[:].rearrange("p (h w) -> p h w", h=HH)[
                    :, r0 + 1:r0 + 3, 1:257]
                nc.tensor.matmul(ps1[32 * k:32 * k + 32, :], saw1t[:, :], yv,
                                 start=True, stop=True,
                                 tile_position=(0, 32 * k)).annotate("mm_sa")
            if g % 2 == 0:
                nc.vector.tensor_scalar_max(s1[:, 512 * g:512 * g + 512], ps1[:, :], 0.0)
            else:
                nc.scalar.activation(s1[:, 512 * g:512 * g + 512], ps1[:, :], AF.Relu)
        for g in range(4):
            ps2 = psA.tile([128, 512], FP32, tag="a")
            for k in range(4):
                nc.tensor.matmul(ps2[32 * k:32 * k + 32, :],
                                 w2rep[32 * k:32 * k + 16, :],
                                 s1[32 * k:32 * k + 16, 512 * g:512 * g + 512],
                                 start=True, stop=True,
                                 tile_position=(32 * k, 32 * k))
            if g % 2 == 0:
                nc.vector.tensor_scalar_max(s2[:, 512 * g:512 * g + 512], ps2[:, :], 0.0)
            else:
                nc.scalar.activation(s2[:, 512 * g:512 * g + 512], ps2[:, :], AF.Relu)
        saT_ps = psQK.tile([128, 64], FP32, tag="qk")
        for j in range(NCH_T):
            nn, off = j // 4, (j % 4) * 128
            g, k = nn // 4, nn % 4
            nc.tensor.matmul(saT_ps[:, j:j + 1],
                             s2[32 * k:32 * k + 16,
                                512 * g + off:512 * g + off + 128],
                             w3rep[32 * k:32 * k + 16, :],
                             start=True, stop=True, tile_position=(32 * k, 0))
        saT = mpool.tile([128, 64], FP32, tag="saT")
        nc.scalar.activation(saT[:], saT_ps[:], AF.Sigmoid)
        return saT

    def v1x1(b, xt):
        """v_pre = Wv @ x over the halo'd band -> [128, FREE] fp16"""
        vp = vppool.tile([128, FREE], FP16, tag="vp")
        c0 = 0
        while c0 < FREE:
            w = min(512, FREE - c0)
            pv = psA.tile([128, 512], FP32, tag="a")
            nc.tensor.matmul(pv[:, 0:w], wv1x1[:, :], xt[:, c0:c0 + w],
                             start=True, stop=True).annotate("mm_v1x1")
            nc.scalar.copy(vp[:, c0:c0 + w], pv[:, 0:w]).annotate("cp_vp")
            c0 += w
        return vp

    def vsum_side(b, vp):
        """vsum[c] = sum over band of v (exact, via window sums of v_pre).

        Returns (vsum tile, filler closures). The closures do the actual
        work (ACT row sums + small DVE combines) and must be popped before
        stage_stats(b) runs."""
        vv = vp[:].rearrange("p (h w) -> p h w", h=HH)

        def edge(k):
            return vv[:, :, k:k + 1].rearrange("p h w -> p (h w)")

        fr = mpool.tile([128, 34], FP32, tag=f"fr{b}")
        junkr = mpool.tile([128, 258], FP16, tag="junkr")
        vsum = mpool.tile([128, 1], FP32, tag=f"vsum{b}")
        closures = []

        def rows(r0, r1):
            def f():
                for r in range(r0, r1):
                    rowap = vv[:, r:r + 1, :].rearrange("p h w -> p (h w)")
                    nc.scalar.activation(junkr[:], rowap, AF.Copy,
                                         accum_out=fr[:, r:r + 1]).annotate("vsum_red")
            return f

        for r0 in range(0, HH, 6):
            closures.append(rows(r0, min(r0 + 6, HH)))

        def mini():
            # rs block tj at cols [34*tj:34*tj+34]: row sums over tj..tj+255
            rs = mpool.tile([128, 102], FP32, tag="rs")
            pairs = [(256, 257), (0, 257), (0, 1)]
            for tj, (ka, kb) in enumerate(pairs):
                nc.vector.tensor_tensor(rs[:, 34 * tj:34 * tj + 34], fr[:],
                                        edge(ka), ALU.subtract)
                nc.vector.tensor_tensor(rs[:, 34 * tj:34 * tj + 34],
                                        rs[:, 34 * tj:34 * tj + 34],
                                        edge(kb), ALU.subtract)
            rs3 = rs[:].rearrange("p (t r) -> p t r", t=3)
            tj_tot = mpool.tile([128, 3], FP32, tag="tjt")
            nc.vector.tensor_reduce(tj_tot[:], rs3, mybir.AxisListType.X, ALU.add)
            ex = [(32, 33), (0, 33), (0, 1)]
            ws = mpool.tile([128, 9], FP32, tag="ws")
            for ti in range(3):
                a_, b_ = ex[ti]
                ra = rs3[:, :, a_:a_ + 1].rearrange("p t r -> p (t r)")
                rb = rs3[:, :, b_:b_ + 1].rearrange("p t r -> p (t r)")
                nc.vector.tensor_tensor(ws[:, 3 * ti:3 * ti + 3], tj_tot[:],
                                        ra, ALU.subtract)
                nc.vector.tensor_tensor(ws[:, 3 * ti:3 * ti + 3],
                                        ws[:, 3 * ti:3 * ti + 3],
                                        rb, ALU.subtract)
            wsw = mpool.tile([128, 9], FP32, tag="wsw")
            nc.vector.tensor_tensor(wsw[:], ws[:], dwvw[:], ALU.mult)
            nc.vector.tensor_reduce(vsum[:], wsw[:], mybir.AxisListType.X,
                                    ALU.add)

        closures.append(mini)
        return vsum, closures

    def qk_gram(b, xt, saT, fillers):
        """QK 9-tap fused conv in transposed layout + gram accumulation."""
        G = psG.tile([128, 256], FP32, tag="G")
        G2 = psG.tile([128, 128], FP32, tag="G2")
        for j in range(NCH_T):
            if j % FILL_EVERY == FILL_EVERY - 1 and fillers:
                fillers.popleft()()
            r, c0 = j // 2, (j % 2) * 128
            pqk = psQK.tile([128, 256], FP32, tag="qk")
            for t in range(9):
                ti, tj = t // 3, t % 3
                base = (r + ti) * WW + c0 + tj
                nc.tensor.matmul(pqk[:, :], xt[:, base:base + 128],
                                 w9qk[:, 256 * t:256 * t + 256],
                                 start=(t == 0), stop=(t == 8)).annotate("mm_qk")
            rt = rpool.tile([128, 256], BF16, tag="ring")
            nc.vector.tensor_scalar_mul(rt[:, 0:128], pqk[:, 0:128],
                                        saT[:, j:j + 1]).annotate("cp_rtq")
            nc.scalar.copy(rt[:, 128:256], pqk[:, 128:256]).annotate("cp_rtk")
            nc.tensor.matmul(G[:, 0:256], rt[:, 0:128], rt[:, 0:256],
                             start=(j == 0), stop=(j == NCH_T - 1),
                             skip_group_check=True).annotate("mm_gram")
            nc.tensor.matmul(G2[:, :], rt[:, 128:256], rt[:, 128:256],
                             start=(j == 0), stop=(j == NCH_T - 1),
                             skip_group_check=True).annotate("mm_gram")
        return G, G2

    def stage_stats(b, G, G2, vsum):
        """arst [128, 131]: [Gqk | qd | kd | vsum]"""
        arst = mpool.tile([128, 131], FP32, tag=f"arst{b}")
        junk = mpool.tile([128, 128], FP32, tag="junk")
        nc.vector.tensor_copy(arst[:, 0:128], G[:, 128:256])
        nc.vector.scalar_tensor_tensor(junk[:], G[:, 0:128], 1.0, eye,
                                       ALU.mult, ALU.mult,
                                       accum_out=arst[:, 128:129])
        nc.vector.scalar_tensor_tensor(junk[:], G2[:, :], 1.0, eye,
                                       ALU.mult, ALU.mult,
                                       accum_out=arst[:, 129:130])
        nc.vector.tensor_copy(arst[:, 130:131], vsum[:])
        return arst

    def issue_ar(b, arst):
        din = dpool.tile([128, 131], FP32, tag=f"din{b}")
        dout = dpool.tile([128, 131], FP32, tag=f"dout{b}")
        dma(din[:], arst[:])
        nc.gpsimd.collective_compute(
            "AllReduce", ALU.add,
            replica_groups=[list(range(NCORES))],
            ins=[din[:].opt()], outs=[dout[:].opt()])
        arres = mpool.tile([128, 131], FP32, tag=f"arres{b}")
        dma(arres[:], dout[:])
        return arres

    def tap_fillers(spec, src, dwv, acc_pool, acc_tag):
        """depthwise 3x3 over halo'd src -> acc [128, NLOC] fp16.

        Returns (acc, te_taps, fillers): each filler is a closure emitting
        one half-width tap op pair; pop them in order."""
        sv = src[:].rearrange("p (h w) -> p h w", h=HH)
        acc = acc_pool.tile([128, NLOC], FP16, tag=acc_tag)
        d_taps = [t for t in range(9) if spec[t] == "d"]
        g_taps = [t for t in range(9) if spec[t] == "g"]
        te_taps = [t for t in range(9) if spec[t] == "t"]
        f_taps = [t for t in range(9) if spec[t] == "f"]

        def shifted(t, c0, w):
            ti, tj = t // 3, t % 3
            r0, cw = c0 // W, c0 % W
            nr = w // W
            return sv[:, ti + r0:ti + r0 + nr, tj + cw:tj + cw + W]

        fillers = []
        for i, t in enumerate(d_taps):
            for h in range(NLOC // TMPH):
                def f(t=t, h=h, first=(i == 0)):
                    c0 = h * TMPH
                    dst = acc[:, c0:c0 + TMPH]
                    dstv = dst.rearrange("p (h w) -> p h w", h=TMPH // W)
                    if first:
                        nc.vector.tensor_scalar_mul(
                            dstv, shifted(t, c0, TMPH),
                            dwv[:, t:t + 1]).annotate("tap_ts")
                    else:
                        tmp = tmppool.tile([128, TMPH], FP16, tag="tmp")
                        tmpv = tmp[:].rearrange("p (h w) -> p h w", h=TMPH // W)
                        nc.vector.tensor_scalar_mul(
                            tmpv, shifted(t, c0, TMPH),
                            dwv[:, t:t + 1]).annotate("tap_ts")
                        nc.vector.tensor_tensor(dst, dst, tmp[:],
                                                ALU.add).annotate("tap_tt")
                fillers.append(f)
        for t in g_taps:
            for h in range(NLOC // TMPH):
                def f(t=t, h=h):
                    for q in range(TMPH // TMPG):
                        c0 = h * TMPH + q * TMPG
                        dst = acc[:, c0:c0 + TMPG]
                        tmpg = tmppool.tile([128, TMPG], FP16, tag="tmpg")
                        tmpgv = tmpg[:].rearrange("p (h w) -> p h w",
                                                  h=TMPG // W)
                        nc.vector.tensor_scalar_mul(
                            tmpgv, shifted(t, c0, TMPG),
                            dwv[:, t:t + 1]).annotate("tap_gts")
                        nc.gpsimd.tensor_tensor(dst, dst, tmpg[:],
                                                ALU.add).annotate("tap_gtt")
                fillers.append(f)
        return acc, te_taps, f_taps, fillers

    def taps_te(b, src, w9, te_taps, acc, merge):
        """TE dense-fold taps over halo'd src, baseline-style; merge into acc."""
        if not te_taps:
            return
        sv = src[:].rearrange("p (h w) -> p h w", h=HH)
        for nn in range(NCH_A):
            r0 = 2 * nn
            py = psA.tile([128, 512], FP32, tag="a")
            for i, t in enumerate(te_taps):
                ti, tj = t // 3, t % 3
                xv = sv[:, r0 + ti:r0 + ti + 2, tj:tj + 256]
                nc.tensor.matmul(py[:, :], w9[:, 128 * t:128 * t + 128], xv,
                                 start=(i == 0),
                                 stop=(i == len(te_taps) - 1)).annotate("mm_yte")
            if merge:
                nc.vector.tensor_tensor(acc[:, 512 * nn:512 * nn + 512],
                                        acc[:, 512 * nn:512 * nn + 512],
                                        py[:, :], ALU.add).annotate("cp_te")
            else:
                nc.scalar.copy(acc[:, 512 * nn:512 * nn + 512],
                               py[:, :]).annotate("cp_te")

    def post_ar(b, arres):
        """norms -> softmax -> Meff/p2t; returns (mefft fp16, p2t fp16)"""
        rqk = mpool.tile([128, 2], FP32, tag="rqk")
        srt = mpool.tile([128, 2], FP32, tag="srt")
        dcat = arres[:, 128:130]
        nc.scalar.activation(srt[:], dcat, AF.Sqrt)
        nc.vector.tensor_scalar_max(srt[:], srt[:], 1e-12)
        nc.vector.reciprocal(rqk[:], srt[:])
        r2 = mpool.tile([128, 2], FP32, tag="r2")
        nc.vector.tensor_tensor(r2[:], rqk[:], rqk[:], ALU.mult)
        nc.vector.tensor_tensor(r2[:], r2[:], dcat, ALU.mult)
        nc.vector.tensor_scalar(r2[:], r2[:], -0.5, 1.5, ALU.mult, ALU.add)
        nc.vector.tensor_tensor(rqk[:], rqk[:], r2[:], ALU.mult)
        rqt = mpool.tile([128, 1], FP32, tag="rqt")
        nc.vector.tensor_tensor(rqt[:], rqk[:, 0:1], tempp, ALU.mult)

        ps1 = psA.tile([128, 128], FP32, tag="a")
        nc.tensor.matmul(ps1[0:1, :], rqk[:, 1:2], eye, start=True, stop=True)
        rkrow = mpool.tile([1, 128], FP32, tag="rkrow")
        nc.scalar.copy(rkrow[:], ps1[0:1, :])
        ps2 = psA.tile([128, 128], FP32, tag="a")
        nc.tensor.matmul(ps2[:, :], onesrow, rkrow[:], start=True, stop=True)

        gh = mpool.tile([128, 128], FP32, tag="gh")
        nc.vector.scalar_tensor_tensor(gh[:], arres[:, 0:128], rqt[:, 0:1],
                                       ps2[:, :], ALU.mult, ALU.mult)
        sm = mpool.tile([128, 128], FP32, tag="sm")
        nc.scalar.activation(sm[:], gh[:], AF.Exp)
        rs_ = mpool.tile([128, 1], FP32, tag="rssm")
        nc.vector.scalar_tensor_tensor(sm[:], sm[:], 1.0, bdmask,
                                       ALU.mult, ALU.mult, accum_out=rs_[:])
        nc.vector.reciprocal(rs_[:], rs_[:])
        attn = mpool.tile([128, 128], FP32, tag="attn")
        nc.vector.tensor_scalar_mul(attn[:], sm[:], rs_[:, 0:1])

        psM = psA.tile([128, 128], FP32, tag="a")
        nc.tensor.matmul(psM[:, :], attn[:], projt[:], start=True, stop=True)
        mefft = mpool.tile([128, 128], FP16, tag="mefft")
        nc.scalar.copy(mefft[:], psM[:, :])

        psT = psA.tile([128, 128], FP32, tag="a")
        nc.tensor.transpose(psT[:, :], attn[:], eye)
        attnt = mpool.tile([128, 128], FP32, tag="attnt")
        nc.vector.tensor_copy(attnt[:], psT[:, :])
        psP = psA.tile([128, 1], FP32, tag="a")
        nc.tensor.matmul(psP[:, :], attnt[:], arres[:, 130:131],
                         start=True, stop=True)
        pooled = mpool.tile([128, 1], FP32, tag="pooled")
        nc.scalar.activation(pooled[:], psP[:, :], AF.Copy, scale=1.0 / NTOT)

        psg1 = psA.tile([16, 1], FP32, tag="a")
        nc.tensor.matmul(psg1[:, :], spw1t[:], pooled[:], start=True, stop=True)
        g1 = mpool.tile([16, 1], FP32, tag="g1")
        nc.scalar.activation(g1[:], psg1[:, :], AF.Gelu)
        psg2 = psA.tile([16, 1], FP32, tag="a")
        nc.tensor.matmul(psg2[:, :], spw2t[:], g1[:], start=True, stop=True)
        g2 = mpool.tile([16, 1], FP32, tag="g2")
        nc.scalar.activation(g2[:], psg2[:, :], AF.Gelu)
        psg3 = psA.tile([128, 1], FP32, tag="a")
        nc.tensor.matmul(psg3[:, :], spw3t[:], g2[:], start=True, stop=True)
        spec = mpool.tile([128, 1], FP32, tag="spec")
        nc.scalar.activation(spec[:], psg3[:, :], AF.Sigmoid)

        p2t = mpool.tile([128, 128], FP16, tag="p2t")
        nc.vector.tensor_scalar_mul(p2t[:], projt[:], spec[:, 0:1])
        return mefft, p2t

    def fold_weights(b, mefft, p2t, vf_taps, yf_taps, xt, yt):
        """lhsT weights for proj-folded taps: K_t = w9vf_t^T @ mefft (v),
        M_t = p2t * dwy_t (y). Returns extras list for final_proj."""
        extras = []
        for t in vf_taps:
            psK = psA.tile([128, 128], FP32, tag="a")
            nc.tensor.matmul(psK[:, :], w9vf[:, 128 * t:128 * t + 128],
                             mefft[:], start=True, stop=True)
            kt = mpool.tile([128, 128], BF16, tag=f"kt{t}")
            nc.scalar.copy(kt[:], psK[:, :])
            extras.append((kt, xt, t))
        for t in yf_taps:
            mt = mpool.tile([128, 128], BF16, tag=f"mt{t}")
            nc.vector.tensor_scalar_mul(mt[:], p2t[:], dwyw[:, t:t + 1])
            extras.append((mt, yt, t))
        return extras

    def final_proj(b, mefft, p2t, vt, y2t, extras=()):
        out2d = io["out"][b].rearrange("c h w -> c (h w)")
        for nn in range(NCH_A):
            r0 = 2 * nn
            pf = psA.tile([128, 512], FP32, tag="a")
            nc.tensor.matmul(pf[:, :], mefft[:],
                             vt[:, 512 * nn:512 * nn + 512],
                             start=True, stop=False).annotate("mm_proj")
            nc.tensor.matmul(pf[:, :], p2t[:],
                             y2t[:, 512 * nn:512 * nn + 512],
                             start=False, stop=len(extras) == 0).annotate("mm_proj")
            for i, (wt, srct, t) in enumerate(extras):
                ti, tj = t // 3, t % 3
                sv = srct[:].rearrange("p (h w) -> p h w", h=HH)
                xv = sv[:, r0 + ti:r0 + ti + 2, tj:tj + 256]
                nc.tensor.matmul(pf[:, :], wt[:], xv, start=False,
                                 stop=i == len(extras) - 1).annotate("mm_projf")
            ot = opool.tile([128, 512], FP16, tag="ot")
            nc.scalar.copy(ot[:], pf[:, :]).annotate("cp_ot")
            dma(out2d[:, 512 * nn:512 * nn + 512], ot[:])

    # ================= schedule =================
    from collections import deque
    F = deque()
    with nc.allow_low_precision(reason="fp16 depthwise accumulation"):
        for b in range(B):
            if b == 0:
                xt, yt = xt0, yt0
            else:
                xt = xpool.tile([128, FREE], BF16, tag="x")
                yt = ypool.tile([128, FREE], BF16, tag="y")
                dma(yt[:], io["yh"][b].rearrange("c h w -> c (h w)"))
                dma(xt[:], io["xh"][b].rearrange("c h w -> c (h w)"))
            xts.append(xt)
            yts.append(yt)

            saT = sa_gate(b, yt)
            saTs.append(saT)
            vp = v1x1(b, xt)
            vsum, fvs = vsum_side(b, vp)
            vt, v_te, vf, fv = tap_fillers(V_TAPS[b], vp, dwvw, vpool, "vt")
            taps_te(b, xt, w9v, v_te, vt, merge=True)
            vts.append(vt)
            vfs.append(vf)
            y2t, y_te, yf, fy = tap_fillers(Y_TAPS[b], yt, dwyw, y2pool, "y2t")
            taps_te(b, yt, wyd, y_te, y2t, merge=True)
            y2ts.append(y2t)
            yfs.append(yf)
            F.extend(fvs)
            F.extend(fv)
            F.extend(fy)
            # a few fillers ahead of the QK stream
            for _ in range(3):
                if F:
                    F.popleft()()
            G, G2 = qk_gram(b, xt, saT, F)
            arst = stage_stats(b, G, G2, vsum)
            arres = issue_ar(b, arst)
            arreses.append(arres)
            if b == 1:
                mp0 = post_ar(0, arreses[0])

        ex0 = fold_weights(0, mp0[0], mp0[1], vfs[0], yfs[0], xts[0], yts[0])
        while F:
            F.popleft()()
        final_proj(0, mp0[0], mp0[1], vts[0], y2ts[0], ex0)
        mefft1, p2t1 = post_ar(1, arreses[1])
        ex1 = fold_weights(1, mefft1, p2t1, vfs[1], yfs[1], xts[1], yts[1])
        final_proj(1, mefft1, p2t1, vts[1], y2ts[1], ex1)

    ctx.close()


def build_nc():
    nc = bacc.Bacc("TRN2", target_bir_lowering=False, debug=False,
                   num_devices=NCORES)
    io = {}

    def inp(name, shape, dt):
        io[name] = nc.dram_tensor(name, shape, dt, kind="ExternalInput")

    inp("xh", [B, C, HH, WW], BF16)
    inp("yh", [B, C, HH, WW], BF16)
    inp("w9qk", [128, 9 * 256], BF16)
    inp("w9v", [128, 9 * 128], BF16)
    inp("w9vf", [128, 9 * 128], FP16)
    inp("wyd", [128, 9 * 128], BF16)
    inp("wv1x1", [128, 128], BF16)
    inp("saw1t", [128, 32], BF16)
    inp("w2rep", [128, 32], BF16)
    inp("w3rep", [128, 1], BF16)
    inp("spw1t", [128, 16], FP32)
    inp("spw2t", [16, 16], FP32)
    inp("spw3t", [16, 128], FP32)
    inp("projt", [128, 128], FP32)
    inp("dwvw", [128, 9], FP32)
    inp("dwyw", [128, 9], FP32)
    inp("consts", [128, 386], FP32)
    io["out"] = nc.dram_tensor("out", [B, C, RPC, W], FP16, kind="ExternalOutput")

    with tile.TileContext(nc) as tc:
        _emit(tc, io)
    nc.finalize()
    return nc


_CACHE = {}


def _prep_host(x, y, qkv_w, qkv_dw_w, proj_w, sa_w1, sa_w2, sa_w3,
               sp_w1, sp_w2, sp_w3, dw_w, temperature):
    import ml_dtypes
    bf = ml_dtypes.bfloat16
    f32 = np.float32

    x = np.asarray(x, f32)
    y = np.asarray(y, f32)
    xp = np.zeros((B, C, H + 2, W + 2), f32)
    xp[:, :, 1:H + 1, 1:W + 1] = x
    yp = np.zeros((B, C, H + 2, W + 2), f32)
    yp[:, :, 1:H + 1, 1:W + 1] = y
    xp = xp.astype(bf)
    yp = yp.astype(bf)

    qkv_w = np.asarray(qkv_w, f32)
    dw = np.asarray(qkv_dw_w, f32).reshape(3 * C, 9)
    w9qk = np.concatenate(
        [(qkv_w[:256] * dw[:256, t:t + 1]).T for t in range(9)], axis=1)
    w9v = np.concatenate(
        [(qkv_w[256:] * dw[256:, t:t + 1]).T for t in range(9)], axis=1)
    w9vf = np.concatenate(
        [(qkv_w[256:] * dw[256:, t:t + 1]) for t in range(9)], axis=1)
    dwy = np.asarray(dw_w, f32).reshape(C, 9)
    wyd = np.concatenate(
        [np.diag(dwy[:, t]) for t in range(9)], axis=1)

    w2rep = np.zeros((128, 32), f32)
    w3rep = np.zeros((128, 1), f32)
    for k in range(4):
        w2rep[32 * k:32 * k + 16, 0:16] = np.asarray(sa_w2, f32).T
        w3rep[32 * k:32 * k + 16] = np.asarray(sa_w3, f32).T
    saw1tp = np.zeros((128, 32), f32)
    saw1tp[:, 0:16] = np.asarray(sa_w1, f32).T

    consts = np.zeros((128, 386), f32)
    consts[:, 0:128] = np.eye(128, dtype=f32)
    ci = np.arange(128) // DH
    consts[:, 128:256] = (ci[:, None] == ci[None, :]).astype(f32)
    consts[:, 256] = np.asarray(temperature, f32).reshape(HD)[ci]
    consts[0, 257:385] = 1.0

    common = {
        "w9qk": w9qk.astype(bf), "w9v": w9v.astype(bf), "wyd": wyd.astype(bf),
        "w9vf": w9vf.astype(np.float16),
        "wv1x1": np.ascontiguousarray(qkv_w[256:].T).astype(bf),
        "saw1t": saw1tp.astype(bf),
        "w2rep": w2rep.astype(bf), "w3rep": w3rep.astype(bf),
        "spw1t": np.asarray(sp_w1, f32).T.copy(),
        "spw2t": np.asarray(sp_w2, f32).T.copy(),
        "spw3t": np.asarray(sp_w3, f32).T.copy(),
        "projt": np.asarray(proj_w, f32).T.copy(),
        "dwvw": np.ascontiguousarray(dw[256:]),
        "dwyw": np.ascontiguousarray(dwy),
        "consts": consts,
    }
    in_maps = []
    for i in range(NCORES):
        m = dict(common)
        m["xh"] = np.ascontiguousarray(xp[:, :, 32 * i:32 * i + HH, :])
        m["yh"] = np.ascontiguousarray(yp[:, :, 32 * i:32 * i + HH, :])
        in_maps.append(m)
    return in_maps


def kernel(**inputs):
    if "nc" not in _CACHE:
        _CACHE["nc"] = build_nc()
    nc = _CACHE["nc"]
    in_maps = _prep_host(**inputs)
    res = run_bass_kernel_spmd(nc, in_maps, core_ids=list(range(NCORES)))
    shards = [res.results[i]["out"] for i in range(NCORES)]
    return np.concatenate(shards, axis=2).astype(np.float32)
